# revision 1
# baseline (speedup 1.0000x reference)
"""Gemma-style sliding-window attention block on 8 trn2 NeuronCores.

Sharding: tensor-parallel over kv-head groups (4) x data-parallel over
batch (2).  Core c handles batch b = c//4 and kv-head g = c%4 (query
heads 2g, 2g+1).  Each core computes its heads' Q/K/V projections,
RMS norms, RoPE, sliding-window attention and the partial Wo
projection; the host sums the 4 partial outputs per batch.

All matmuls run in float32r (fp32 with 11-bit mantissa, full PE rate at
free-dim >= 256).  Host pre-rounds DMA'd operands; on-chip producers
write f32r directly.  Softmax is row-layout (queries on partitions)
with exact row max; attn tiles are PE-transposed for the AV matmul.
"""
import numpy as np
from contextlib import ExitStack

import concourse.bass as bass
import concourse.bacc as bacc
import concourse.mybir as mybir
import concourse.tile as tile
from concourse.bass_utils import run_bass_kernel_spmd

F32 = mybir.dt.float32
F32R = mybir.dt.float32r
AL = mybir.AluOpType
AF = mybir.ActivationFunctionType

B, S, H = 2, 2048, 2560
NH, NKV, D = 8, 4, 256
SW = 1024
EPS = 1e-6
ST = S // 128            # 16 sequence tiles
KT = H // 128            # 20 hidden k-tiles
NSC = S // 512           # 4 sequence chunks
WT = 9                   # window tiles per query tile (1024/128 + 1)
DQ = 2 * D               # per-core query dims (2 heads)
NDQ = DQ // 128          # 4
NDK = D // 128           # 2


def round_f32r(x: np.ndarray) -> np.ndarray:
    """Round fp32 to f32r (11-bit mantissa, round-to-nearest-even)."""
    b = np.ascontiguousarray(x, dtype=np.float32).view(np.uint32).astype(np.uint64)
    bias = 0x7FF + ((b >> 12) & 1)
    return ((b + bias) & 0xFFFFF000).astype(np.uint32).view(np.float32)


def build_nc(debug=False):
    nc = bacc.Bacc("TRN2", target_bir_lowering=False, debug=False)

    hsT_d = nc.dram_tensor("hsT", [KT, 128, S], F32R, kind="ExternalInput")
    wq_d = nc.dram_tensor("wqT", [KT, 128, DQ], F32R, kind="ExternalInput")
    wk_d = nc.dram_tensor("wkT", [KT, 128, D], F32R, kind="ExternalInput")
    wv_d = nc.dram_tensor("wvT", [KT, 128, D], F32R, kind="ExternalInput")
    wo_d = nc.dram_tensor("woT", [NDQ, 128, H], F32R, kind="ExternalInput")
    cos_d = nc.dram_tensor("cosT", [NDK, 128, S], F32, kind="ExternalInput")
    sin_d = nc.dram_tensor("sinT", [NDK, 128, S], F32, kind="ExternalInput")
    msk_d = nc.dram_tensor("masks", [ST, 3, 128, 384], F32R, kind="ExternalInput")
    idn_d = nc.dram_tensor("ident", [128, 128], F32R, kind="ExternalInput")
    ones_d = nc.dram_tensor("ones_c", [128, 1], F32R, kind="ExternalInput")
    onesr_d = nc.dram_tensor("onesr_c", [1, 128], F32R, kind="ExternalInput")
    qw_d = nc.dram_tensor("qw1p", [128, NDK], F32, kind="ExternalInput")
    kw_d = nc.dram_tensor("kw1p", [128, NDK], F32, kind="ExternalInput")
    out_d = nc.dram_tensor("out", [S, H], F32, kind="ExternalOutput")
    dbg = {}
    if debug:
        for nm, shp in [("dQT0", [128, S]), ("dKT0", [128, S]),
                        ("dV", [128, ST * D]), ("dexpb", [128, 1152]),
                        ("dao", [128, D]), ("daoT0", [128, S]),
                        ("dsc", [128, 1152]), ("dnegm", [128, 1])]:
            dbg[nm] = nc.dram_tensor(nm, shp, F32, kind="ExternalOutput")

    with ExitStack() as top:
        tc = top.enter_context(tile.TileContext(nc))
        big = top.enter_context(tc.tile_pool(name="big", bufs=1))

        # Resident tensors (whole-kernel lifetime)
        QT = [big.tile([128, S], F32R, name=f"QT{j}", tag=f"QT{j}") for j in range(NDQ)]
        KTt = [big.tile([128, S], F32R, name=f"KTt{j}", tag=f"KTt{j}") for j in range(NDK)]
        V = big.tile([128, ST, D], F32R, tag="V")
        aoT = [big.tile([128, S], F32R, name=f"aoT{j}", tag=f"aoT{j}") for j in range(NDQ)]
        ident = big.tile([128, 128], F32R, tag="ident")
        ones = big.tile([128, 1], F32R, tag="ones")
        onesr = big.tile([1, 128], F32R, tag="onesr")
        epsb = big.tile([128, 1], F32, tag="epsb")
        qw1p = big.tile([128, NDK], F32, tag="qw1p")
        kw1p = big.tile([128, NDK], F32, tag="kw1p")
        nc.sync.dma_start(out=ident, in_=idn_d[:, :])
        nc.sync.dma_start(out=qw1p, in_=qw_d[:, :])
        nc.sync.dma_start(out=kw1p, in_=kw_d[:, :])
        nc.sync.dma_start(out=ones, in_=ones_d[:, :])
        nc.sync.dma_start(out=onesr, in_=onesr_d[:, :])
        nc.vector.memset(epsb, EPS)

        # ---------------- Phase 1: projections + norms + rope -------------
        with ExitStack() as p1:
            wpool = p1.enter_context(tc.tile_pool(name="wpool", bufs=1))
            wstr = p1.enter_context(tc.tile_pool(name="wstr", bufs=3))
            hpool = p1.enter_context(tc.tile_pool(name="hpool", bufs=3))
            cpool = p1.enter_context(tc.tile_pool(name="cpool", bufs=2))
            tpool = p1.enter_context(tc.tile_pool(name="tpool", bufs=1))
            spool = p1.enter_context(tc.tile_pool(name="spool", bufs=2))
            spool1 = p1.enter_context(tc.tile_pool(name="spool1", bufs=1))
            pps = p1.enter_context(tc.tile_pool(name="pps", bufs=1, space="PSUM"))

            wq = wpool.tile([128, KT, DQ], F32R, tag="wq")
            wk = wpool.tile([128, KT, D], F32R, tag="wk")
            nc.sync.dma_start(out=wq, in_=wq_d.rearrange("k p m -> p k m"))
            nc.sync.dma_start(out=wk, in_=wk_d.rearrange("k p m -> p k m"))

            for sc in range(NSC):
                sl = slice(sc * 512, (sc + 1) * 512)
                qps = [pps.tile([128, 512], F32, name=f"qps{j}", tag=f"qps{j}") for j in range(NDQ)]
                kps = [pps.tile([128, 512], F32, name=f"kps{j}", tag=f"kps{j}") for j in range(NDK)]
                vps = pps.tile([128, 4, D], F32, tag="vps")
                vflat = vps.rearrange("p a b -> p (a b)")

                for kt in range(KT):
                    hst = hpool.tile([128, 512], F32R, tag="hst")
                    nc.sync.dma_start(out=hst, in_=hsT_d[kt, :, sl])
                    wv = wstr.tile([128, D], F32R, tag="wv")
                    nc.sync.dma_start(out=wv, in_=wv_d[kt, :, :])
                    st_, sp_ = (kt == 0), (kt == KT - 1)
                    for j in range(NDQ):
                        nc.tensor.matmul(qps[j], wq[:, kt, j * 128:(j + 1) * 128],
                                         hst, start=st_, stop=sp_)
                    for j in range(NDK):
                        nc.tensor.matmul(kps[j], wk[:, kt, j * 128:(j + 1) * 128],
                                         hst, start=st_, stop=sp_)
                    for i in range(4):
                        # i in {1,3} shares a PSUM bank with i-1; start=True
                        # clears the whole bank, so only the first sub-tile
                        # per bank starts the group (has_written bits make the
                        # sibling's first write an overwrite).
                        nc.tensor.matmul(vps[:, i, :], hst[:, i * 128:(i + 1) * 128],
                                         wv, start=(st_ and i % 2 == 0), stop=sp_)

                # V rms norm (no weight): rows are sequence positions
                for i in range(4):
                    vscr = tpool.tile([128, D], F32, tag="vscr")
                    msq = spool.tile([128, 1], F32, tag="msq")
                    nc.scalar.activation(out=vscr, in_=vps[:, i, :],
                                         func=AF.Square, accum_out=msq)
                    sdv = spool.tile([128, 1], F32, tag="sdv")
                    nc.scalar.activation(out=sdv, in_=msq, func=AF.Sqrt,
                                         scale=1.0 / D, bias=epsb)
                    rv = spool.tile([128, 1], F32, tag="rv")
                    nc.vector.reciprocal(out=rv, in_=sdv)
                    nc.vector.tensor_scalar_mul(V[:, sc * 4 + i, :], vps[:, i, :], rv)

                # Q/K rms norm + rope (transposed layout: d on partitions)
                # heads: (dst tiles, psum tiles, d-tile idx pairs, weight)
                heads = [(QT, qps, (0, 1), qw1p), (QT, qps, (2, 3), qw1p),
                         (KTt, kps, (0, 1), kw1p)]
                cosA = cpool.tile([128, 512], F32, tag="cosA")
                cosB = cpool.tile([128, 512], F32, tag="cosB")
                sinA = cpool.tile([128, 512], F32, tag="sinA")
                sinB = cpool.tile([128, 512], F32, tag="sinB")
                nc.sync.dma_start(out=cosA, in_=cos_d[0, :, sl])
                nc.sync.dma_start(out=cosB, in_=cos_d[1, :, sl])
                nc.sync.dma_start(out=sinA, in_=sin_d[0, :, sl])
                nc.sync.dma_start(out=sinB, in_=sin_d[1, :, sl])
                for hidx, (dst, src, (jA, jB), w1p) in enumerate(heads):
                    ssq_home = vflat[0:1, 0:512] if hidx != 1 else vflat[0:1, 512:1024]
                    sq = [tpool.tile([128, 512], F32R, name=f"sq{j}", tag=f"sq{j}") for j in (0, 1)]
                    for j, jj in enumerate((jA, jB)):
                        nc.scalar.activation(out=sq[j], in_=src[jj], func=AF.Square)
                    nc.tensor.matmul(ssq_home, ones, sq[0], start=True, stop=False)
                    nc.tensor.matmul(ssq_home, ones, sq[1], start=False, stop=True)
                    sd = spool1.tile([1, 512], F32, tag="sd")
                    nc.scalar.activation(out=sd, in_=ssq_home, func=AF.Sqrt,
                                         scale=1.0 / D, bias=epsb[0:1, :])
                    rqf = spool1.tile([1, 512], F32, tag="rqf")
                    nc.vector.reciprocal(out=rqf, in_=sd)
                    # hi/lo split so the f32r rank-1 broadcast is fp32-exact
                    rq = spool1.tile([1, 512], F32R, tag="rq")
                    nc.vector.tensor_copy(out=rq, in_=rqf)
                    rql = spool1.tile([1, 512], F32R, tag="rql")
                    with nc.allow_low_precision(reason="f32r lo residual"):
                        nc.vector.tensor_sub(rql, rqf, rq)
                    bcps = vflat[:, 0:512] if hidx != 1 else vflat[:, 512:1024]
                    nc.tensor.matmul(bcps, onesr, rq, start=True, stop=False)
                    nc.tensor.matmul(bcps, onesr, rql, start=False, stop=True)
                    bc = tpool.tile([128, 512], F32, tag="bc")
                    nc.scalar.copy(out=bc, in_=bcps)
                    qn = []
                    for j, jj in enumerate((jA, jB)):
                        q = tpool.tile([128, 512], F32, name=f"qn{j}", tag=f"qn{j}")
                        nc.vector.scalar_tensor_tensor(
                            out=q, in0=src[jj], scalar=w1p[:, j:j + 1],
                            in1=bc, op0=AL.mult, op1=AL.mult)
                        qn.append(q)
                    t1 = tpool.tile([128, 512], F32, tag="t1")
                    t2 = tpool.tile([128, 512], F32, tag="t2")
                    nc.vector.tensor_mul(t1, qn[0], cosA)
                    nc.vector.tensor_mul(t2, qn[1], sinA)
                    nc.vector.tensor_sub(dst[jA][:, sl], t1, t2)
                    t3 = tpool.tile([128, 512], F32, tag="t1")
                    t4 = tpool.tile([128, 512], F32, tag="t2")
                    nc.vector.tensor_mul(t3, qn[1], cosB)
                    nc.vector.tensor_mul(t4, qn[0], sinB)
                    nc.vector.tensor_add(dst[jB][:, sl], t3, t4)

        if debug:
            nc.sync.dma_start(out=dbg["dQT0"][:, :], in_=QT[0].bitcast(F32))
            nc.sync.dma_start(out=dbg["dKT0"][:, :], in_=KTt[0].bitcast(F32))
            nc.sync.dma_start(out=dbg["dV"][:, :],
                              in_=V.rearrange("p a b -> p (a b)").bitcast(F32))

        # ---------------- Phase 2: attention ------------------------------
        with ExitStack() as p23:
            wopool = p23.enter_context(tc.tile_pool(name="wopool", bufs=1))
            p2 = p23.enter_context(ExitStack())
            mpool = p2.enter_context(tc.tile_pool(name="mpool", bufs=2))
            epool = p2.enter_context(tc.tile_pool(name="epool", bufs=3))
            npool = p2.enter_context(tc.tile_pool(name="npool", bufs=3))
            scps = p2.enter_context(tc.tile_pool(name="scps", bufs=1, space="PSUM"))
            trps = p2.enter_context(tc.tile_pool(name="trps", bufs=2, space="PSUM"))
            aops_p = p2.enter_context(tc.tile_pool(name="aops", bufs=3, space="PSUM"))

            woT = wopool.tile([128, NDQ, H], F32R, tag="woT")
            nc.sync.dma_start(out=woT, in_=wo_d.rearrange("k p m -> p k m"))

            dbg_sc_sb = (epool.tile([128, 1152], F32, name="dbgsc", tag="dbgsc")
                         if debug else None)
            for t in range(ST):
                w0 = max(0, t - 8)
                msk = mpool.tile([128, 3, 384], F32R, tag="msk")
                nc.sync.dma_start(out=msk,
                                  in_=msk_d[t].rearrange("c p n -> p c n"))
                mask_chunks = (0, 1, 2) if t < 8 else (0, 2)
                for h in range(2):
                    scs = [scps.tile([128, 512], F32, name=f"sc{c}", tag=f"sc{c}")[:, :384]
                           for c in range(3)]
                    for c in range(3):
                        has_mask = c in mask_chunks
                        rhs_sl = slice(w0 * 128 + c * 384, w0 * 128 + c * 384 + 384)
                        for j in range(NDK):
                            nc.tensor.matmul(
                                scs[c], QT[2 * h + j][:, t * 128:(t + 1) * 128],
                                KTt[j][:, rhs_sl], start=(j == 0),
                                stop=(j == 1 and not has_mask))
                        if has_mask:
                            nc.tensor.matmul(scs[c], ident, msk[:, c, :],
                                             start=False, stop=True)
                    nm = [npool.tile([128, 1], F32, name=f"nm{c}", tag=f"nm{c}") for c in range(3)]
                    for c in range(3):
                        nc.vector.tensor_reduce(out=nm[c], in_=scs[c],
                                                axis=mybir.AxisListType.X,
                                                op=AL.max, negate=True)
                    negm = npool.tile([128, 1], F32, tag="negm")
                    nc.vector.tensor_tensor(negm, nm[0], nm[1], op=AL.min)
                    nc.vector.tensor_tensor(negm, negm, nm[2], op=AL.min)
                    expb = epool.tile([128, 1152], F32R, tag="expb")
                    den = npool.tile([128, 3], F32, tag="den")
                    for c in range(3):
                        nc.scalar.activation(out=expb[:, c * 384:(c + 1) * 384],
                                             in_=scs[c], func=AF.Exp, bias=negm,
                                             accum_out=den[:, c:c + 1])
                    dsum = npool.tile([128, 1], F32, tag="dsum")
                    nc.vector.tensor_reduce(out=dsum, in_=den,
                                            axis=mybir.AxisListType.X, op=AL.add)
                    rden = npool.tile([128, 1], F32, tag="rden")
                    nc.vector.reciprocal(out=rden, in_=dsum)

                    if debug and t == 10 and h == 0:
                        nc.sync.dma_start(out=dbg["dexpb"][:, :],
                                          in_=expb.bitcast(F32))
                        nc.sync.dma_start(out=dbg["dnegm"][:, :], in_=negm)
                        for c in range(3):
                            nc.vector.tensor_copy(
                                out=dbg_sc_sb[:, c * 384:(c + 1) * 384],
                                in_=scs[c])
                        nc.sync.dma_start(out=dbg["dsc"][:, :], in_=dbg_sc_sb)
                    expT = epool.tile([128, WT, 128], F32R, tag="expT")
                    for a in range(WT):
                        trp = trps.tile([128, 128], F32R, tag="trp")
                        nc.tensor.transpose(trp, expb[:, a * 128:(a + 1) * 128],
                                            ident)
                        if a % 2 == 0:
                            nc.vector.tensor_copy(out=expT[:, a, :], in_=trp)
                        else:
                            nc.scalar.copy(out=expT[:, a, :], in_=trp)
                    aop = aops_p.tile([128, D], F32, tag="aop")
                    for a in range(WT):
                        nc.tensor.matmul(aop, expT[:, a, :], V[:, w0 + a, :],
                                         start=(a == 0), stop=(a == WT - 1))
                    ao = epool.tile([128, D], F32R, tag="ao")
                    nc.scalar.activation(out=ao, in_=aop, func=AF.Copy, scale=rden)
                    if debug and t == 10 and h == 0:
                        nc.sync.dma_start(out=dbg["dao"][:, :],
                                          in_=ao.bitcast(F32))
                    for j in range(2):
                        trp = trps.tile([128, 128], F32R, tag="trp")
                        nc.tensor.transpose(trp, ao[:, j * 128:(j + 1) * 128], ident)
                        nc.vector.tensor_copy(
                            out=aoT[2 * h + j][:, t * 128:(t + 1) * 128], in_=trp)

            if debug:
                nc.sync.dma_start(out=dbg["daoT0"][:, :], in_=aoT[0].bitcast(F32))
            # ------------- Phase 3: output projection ---------------------
            p2.close()
            with ExitStack() as p3:
                opool = p3.enter_context(tc.tile_pool(name="opool", bufs=3))
                wops = p3.enter_context(tc.tile_pool(name="wops", bufs=2,
                                                     space="PSUM"))
                for st_i in range(ST):
                    for hc in range(H // 512):
                        wop = wops.tile([128, 512], F32, tag="wop")
                        for dj in range(NDQ):
                            nc.tensor.matmul(
                                wop, aoT[dj][:, st_i * 128:(st_i + 1) * 128],
                                woT[:, dj, hc * 512:(hc + 1) * 512],
                                start=(dj == 0), stop=(dj == NDQ - 1))
                        osb = opool.tile([128, 512], F32, tag="osb")
                        if (st_i + hc) % 2 == 0:
                            nc.vector.tensor_copy(out=osb, in_=wop)
                        else:
                            nc.scalar.copy(out=osb, in_=wop)
                        nc.sync.dma_start(
                            out=out_d[st_i * 128:(st_i + 1) * 128,
                                      hc * 512:(hc + 1) * 512], in_=osb)

    nc.compile()
    return nc


_nc_cache = None


def kernel(hidden_states, attention_mask, cos, sin, Wq, Wk, Wv, Wo,
           q_norm_w, k_norm_w):
    global _nc_cache
    if _nc_cache is None:
        _nc_cache = build_nc()
    nc = _nc_cache

    hidden_states = np.asarray(hidden_states, dtype=np.float32)
    mask = np.asarray(attention_mask, dtype=np.float32)[0, 0]      # [S, S]
    cos2 = np.asarray(cos, dtype=np.float32)[0, 0]                 # [S, D]
    sin2 = np.asarray(sin, dtype=np.float32)[0, 0]
    Wq = np.asarray(Wq, dtype=np.float32)
    Wk = np.asarray(Wk, dtype=np.float32)
    Wv = np.asarray(Wv, dtype=np.float32)
    Wo = np.asarray(Wo, dtype=np.float32)

    cosT = np.ascontiguousarray(cos2.T.reshape(NDK, 128, S))
    sinT = np.ascontiguousarray(sin2.T.reshape(NDK, 128, S))

    # Row-layout mask tiles: for query tile t, key chunks of 384 starting
    # at 128*max(0, t-8).
    masks = np.zeros((ST, 3, 128, 384), dtype=np.float32)
    for t in range(ST):
        w0 = max(0, t - 8)
        rows = slice(t * 128, (t + 1) * 128)
        for c in range(3):
            cols = slice(w0 * 128 + c * 384, w0 * 128 + c * 384 + 384)
            masks[t, c] = mask[rows, cols]
    masks = round_f32r(masks)
    ident = round_f32r(np.eye(128, dtype=np.float32))

    in_maps = []
    for core in range(8):
        b, g = core // 4, core % 4
        hsT = round_f32r(np.ascontiguousarray(
            hidden_states[b].T).reshape(KT, 128, S))
        wqT = round_f32r(np.ascontiguousarray(
            Wq[2 * g * D:(2 * g + 2) * D].T).reshape(KT, 128, DQ))
        wkT = round_f32r(np.ascontiguousarray(
            Wk[g * D:(g + 1) * D].T).reshape(KT, 128, D))
        wvT = round_f32r(np.ascontiguousarray(
            Wv[g * D:(g + 1) * D].T).reshape(KT, 128, D))
        woT = round_f32r(np.ascontiguousarray(
            Wo[:, 2 * g * D:(2 * g + 2) * D].T).reshape(NDQ, 128, H))
        qw1p = np.ascontiguousarray(
            (1.0 + np.asarray(q_norm_w, dtype=np.float32)).reshape(NDK, 128).T)
        kw1p = np.ascontiguousarray(
            (1.0 + np.asarray(k_norm_w, dtype=np.float32)).reshape(NDK, 128).T)
        in_maps.append({
            "hsT": hsT, "wqT": wqT, "wkT": wkT, "wvT": wvT, "woT": woT,
            "cosT": cosT, "sinT": sinT, "masks": masks, "ident": ident,
            "ones_c": np.ones((128, 1), dtype=np.float32),
            "onesr_c": np.ones((1, 128), dtype=np.float32),
            "qw1p": qw1p, "kw1p": kw1p,
        })

    res = run_bass_kernel_spmd(nc, in_maps, core_ids=list(range(8)))
    outs = [r["out"] for r in res.results]
    final = np.zeros((B, S, H), dtype=np.float32)
    for core in range(8):
        b = core // 4
        final[b] += outs[core]
    return final



# revision 2
# speedup vs baseline: 1.0434x; 1.0434x over previous
"""Gemma-style sliding-window attention block on 8 trn2 NeuronCores, v2.

Sharding: tensor-parallel over kv-head groups (4) x data-parallel over
batch (2).  Core c handles batch b = c//4 and kv-head g = c%4 (query
heads 2g, 2g+1).  Host sums the 4 partial Wo outputs per batch.

Structure:
- Phase 1 runs in 8 seq-chunks of 256 with a double-buffered 4-bank PSUM
  set; the first two chunks share a merged kt loop paced by the initial
  weight-stream DMAs.  cos/sin chunk loads are issued from the ACT queue
  so their pool-reuse waits cannot stall the SP DMA stream.
- RMS-norm partition sums and all rope multiplies run on gpsimd (SBUF
  only; the BIR verifier forbids gpsimd PSUM access).
- Attention skips fully-masked future key tiles for t<8 and initializes
  score PSUM banks with constant mask tiles (mask-first matmul).
- Attention rounds for t=0..5 run from a small early PSUM pool that
  coexists with the last projection chunk, overlapping its epilogue;
  their attn-out transposes are caught up afterwards.
- Wo projection matmuls are interleaved into the attention loop as PE
  filler work, drained from a backlog queue.
"""
import numpy as np
from contextlib import ExitStack

import concourse.bass as bass
import concourse.bacc as bacc
import concourse.mybir as mybir
import concourse.tile as tile
from concourse import bass_isa
from concourse.bass_utils import run_bass_kernel_spmd

F32 = mybir.dt.float32
F32R = mybir.dt.float32r
BF16 = mybir.dt.bfloat16
AL = mybir.AluOpType
AF = mybir.ActivationFunctionType
AX = mybir.AxisListType

B, S, H = 2, 2048, 2560
NH, NKV, D = 8, 4, 256
SW = 1024
EPS = 1e-6
ST = S // 128             # 16 q/k tiles
KT = H // 128             # 20 hidden k-tiles
NC256 = S // 256          # 8 seq chunks (phase 1)
DQ = 2 * D                # per-core query dims (2 heads)
NDQ = DQ // 128           # 4
NDK = D // 128            # 2
NEARLY = 3                # q-tiles whose attn-out/Wo work is deferred past
                          # the phase-1 SBUF pool barrier

MK_DIAG, MK_BOUND, MK_FULL = 0, 1, 2


def round_f32r(x: np.ndarray) -> np.ndarray:
    """Round fp32 to f32r (11-bit mantissa, round-to-nearest-even)."""
    b = np.ascontiguousarray(x, dtype=np.float32).view(np.uint32).astype(np.uint64)
    bias = 0x7FF + ((b >> 12) & 1)
    return ((b + bias) & 0xFFFFF000).astype(np.uint32).view(np.float32)


def chunk_plan(t):
    """(w0, [ntile per chunk], pad) for q-tile t. Chunks are 2-3 tiles so
    every matmul free dim >= 256 (full f32r rate); t=0 pads one dead tile."""
    w0 = max(0, t - 8)
    wt = min(t, 8) + 1
    plans = {1: [2], 2: [2], 3: [3], 4: [2, 2], 5: [3, 2], 6: [3, 3],
             7: [3, 2, 2], 8: [3, 3, 2], 9: [3, 3, 3]}
    ch = plans[wt]
    return w0, ch, sum(ch) - wt


def build_nc(debug=False):
    nc = bacc.Bacc("TRN2", target_bir_lowering=False, debug=False)

    hs_d = nc.dram_tensor("hsC", [NC256, 128, KT, 256], F32R, kind="ExternalInput")
    wq_d = nc.dram_tensor("wqh", [128, KT, DQ], F32R, kind="ExternalInput")
    wk_d = nc.dram_tensor("wkh", [128, KT, D], F32R, kind="ExternalInput")
    wv_d = nc.dram_tensor("wvh", [128, KT, D], F32R, kind="ExternalInput")
    wo_d = nc.dram_tensor("woh", [128, NDQ, H], F32R, kind="ExternalInput")
    cos_d = nc.dram_tensor("cosh", [128, NDK, S], F32, kind="ExternalInput")
    sin_d = nc.dram_tensor("sinh", [128, NDK, S], F32, kind="ExternalInput")
    m3_d = nc.dram_tensor("m3h", [128, 3, 128], BF16, kind="ExternalInput")
    idn_d = nc.dram_tensor("ident", [128, 128], F32R, kind="ExternalInput")
    idb_d = nc.dram_tensor("identb", [128, 128], BF16, kind="ExternalInput")
    qw_d = nc.dram_tensor("qw1p", [128, NDK], F32, kind="ExternalInput")
    kw_d = nc.dram_tensor("kw1p", [128, NDK], F32, kind="ExternalInput")
    out_d = nc.dram_tensor("out", [S, H], F32, kind="ExternalOutput")
    dbg = {}
    if debug:
        for nm, shp in [("dQT0", [128, S]), ("dKT0", [128, S]),
                        ("dV", [128, ST * D]), ("dexpb", [128, 1152]),
                        ("dnegm", [128, 1]), ("dden", [128, 3])]:
            dbg[nm] = nc.dram_tensor(nm, shp, F32, kind="ExternalOutput")

    with ExitStack() as top:
        tc = top.enter_context(tile.TileContext(nc))
        big = top.enter_context(tc.tile_pool(name="big", bufs=1))

        # Whole-kernel residents
        QT = [big.tile([128, S], F32R, name=f"QT{j}", tag=f"QT{j}") for j in range(NDQ)]
        KTt = [big.tile([128, S], F32R, name=f"KTt{j}", tag=f"KTt{j}") for j in range(NDK)]
        V = big.tile([128, ST, D], F32R, tag="V")
        ident = big.tile([128, 128], F32R, tag="ident")
        identb = big.tile([128, 128], BF16, tag="identb")
        m3 = big.tile([128, 3, 128], BF16, tag="m3")
        epsb = big.tile([128, 1], F32, tag="epsb")
        qw1p = big.tile([128, NDK], F32, tag="qw1p")
        kw1p = big.tile([128, NDK], F32, tag="kw1p")
        # boundary-overlap staging: attn-out and P/P^T tiles for t<NEARLY
        aoE = [big.tile([128, D], F32R, name=f"aoE{i}", tag=f"aoE{i}")
               for i in range(2 * NEARLY)]
        expTE = big.tile([128, 3, 128], F32R, name="expTE", tag="expTE")

        # ---------------- Phase 1: projections + norms + rope -------------
        pA = ExitStack()        # phase-1 SBUF pools
        wpool = pA.enter_context(tc.tile_pool(name="wpool", bufs=1))
        hpool = pA.enter_context(tc.tile_pool(name="hpool", bufs=5))
        cpool = pA.enter_context(tc.tile_pool(name="cpool", bufs=2))
        tpool = pA.enter_context(tc.tile_pool(name="tpool", bufs=1))
        spool = pA.enter_context(tc.tile_pool(name="spool", bufs=2))
        pP = ExitStack()        # phase-1 PSUM (chunks 0..6)
        pps = pP.enter_context(tc.tile_pool(name="pps", bufs=2, space="PSUM"))

        wq = wpool.tile([128, KT, DQ], F32R, tag="wq")
        wk = wpool.tile([128, KT, D], F32R, tag="wk")
        wv = wpool.tile([128, KT, D], F32R, tag="wv")

        # hs chunk tiles in 5-kt quarters
        hquarts = {}

        def load_hq(c, q):
            ht = hpool.tile([128, 5, 256], F32R, name="hst", tag="hst")
            nc.sync.dma_start(out=ht, in_=hs_d[c, :, q * 5:(q + 1) * 5, :])
            hquarts[(c, q)] = ht

        def get_hst(c, kt):
            return hquarts[(c, kt // 5)][:, kt % 5, :]

        def free_hq(c, q):
            hquarts.pop((c, q), None)

        # staged startup: tiny first slivers so matmuls start at ~4us.
        nc.sync.dma_start(out=wq[:, 0:1, :], in_=wq_d[:, 0:1, :])
        nc.sync.dma_start(out=wk[:, 0:1, :], in_=wk_d[:, 0:1, :])
        nc.sync.dma_start(out=wv[:, 0:1, :], in_=wv_d[:, 0:1, :])
        load_hq(0, 0)
        load_hq(1, 0)
        nc.sync.dma_start(out=wq[:, 1:4, :], in_=wq_d[:, 1:4, :])
        nc.sync.dma_start(out=wk[:, 1:6, :], in_=wk_d[:, 1:6, :])
        nc.sync.dma_start(out=wv[:, 1:6, :], in_=wv_d[:, 1:6, :])
        nc.sync.dma_start(out=wq[:, 4:7, :], in_=wq_d[:, 4:7, :])
        load_hq(0, 1)
        load_hq(1, 1)
        nc.sync.dma_start(out=wq[:, 7:10, :], in_=wq_d[:, 7:10, :])
        nc.sync.dma_start(out=wk[:, 6:13, :], in_=wk_d[:, 6:13, :])
        nc.sync.dma_start(out=wv[:, 6:13, :], in_=wv_d[:, 6:13, :])
        load_hq(0, 2)
        nc.sync.dma_start(out=wq[:, 10:14, :], in_=wq_d[:, 10:14, :])
        load_hq(1, 2)
        nc.sync.dma_start(out=wk[:, 13:20, :], in_=wk_d[:, 13:20, :])
        nc.sync.dma_start(out=wv[:, 13:20, :], in_=wv_d[:, 13:20, :])
        load_hq(0, 3)
        nc.sync.dma_start(out=wq[:, 14:17, :], in_=wq_d[:, 14:17, :])
        load_hq(1, 3)
        nc.sync.dma_start(out=wq[:, 17:20, :], in_=wq_d[:, 17:20, :])
        nc.sync.dma_start(out=ident, in_=idn_d[:, :])
        nc.sync.dma_start(out=identb, in_=idb_d[:, :])
        nc.sync.dma_start(out=m3, in_=m3_d[:, :, :])
        nc.sync.dma_start(out=qw1p, in_=qw_d[:, :])
        nc.sync.dma_start(out=kw1p, in_=kw_d[:, :])
        nc.vector.memset(epsb, EPS)

        def proj_matmuls(ps, c, kt):
            hst = get_hst(c, kt)
            st_, sp_ = (kt == 0), (kt == KT - 1)
            for j in range(NDQ):
                nc.tensor.matmul(ps[:, j, :], wq[:, kt, j * 128:(j + 1) * 128],
                                 hst, start=(st_ and j % 2 == 0), stop=sp_)
            for j in range(NDK):
                nc.tensor.matmul(ps[:, 4 + j, :], wk[:, kt, j * 128:(j + 1) * 128],
                                 hst, start=(st_ and j == 0), stop=sp_)
            for i in range(2):
                nc.tensor.matmul(ps[:, 6 + i, :], hst[:, i * 128:(i + 1) * 128],
                                 wv[:, kt, :], start=(st_ and i == 0), stop=sp_)

        def epilogue_stage(ps, c):
            """All ps readers up front so the PSUM set releases quickly."""
            msqs = []
            for i in range(2):
                msq = spool.tile([128, 1], F32, name=f"msq{i}", tag=f"msq{i}")
                vsq = tpool.tile([128, D], F32, name="vsq", tag="hA0")
                nc.scalar.activation(out=vsq, in_=ps[:, 6 + i, :],
                                     func=AF.Square, accum_out=msq)
                msqs.append(msq)
            psts = []
            for gi in range(3):
                pst = tpool.tile([128, 2, 256], F32, name=f"pst{gi}",
                                 tag=f"pst{gi}")
                nc.scalar.copy(out=pst, in_=ps[:, 2 * gi:2 * gi + 2, :])
                psts.append(pst)
            for i in range(2):
                sdv = spool.tile([128, 1], F32, tag="sdv")
                nc.scalar.activation(out=sdv, in_=msqs[i], func=AF.Sqrt,
                                     scale=1.0 / D, bias=epsb)
                rvv = spool.tile([128, 1], F32, tag="rvv")
                nc.vector.reciprocal(out=rvv, in_=sdv)
                nc.scalar.activation(out=V[:, 2 * c + i, :], in_=ps[:, 6 + i, :],
                                     func=AF.Copy, scale=rvv)
            return psts

        def epilogue_heads(c, csn, psts):
            """Norm/rope chain from the SBUF staging tiles.

            Breadth-first emission: same-stage ops across the three head
            groups go out together so the in-order engine queues execute
            the groups concurrently instead of serializing the chains.
            """
            sl = slice(c * 256, (c + 1) * 256)
            heads = [(QT[0], QT[1], 0, qw1p), (QT[2], QT[3], 1, qw1p),
                     (KTt[0], KTt[1], 2, kw1p)]
            slA, slB, slC, slD = [
                [tpool.tile([128, 256], F32, name=f"{nm}{g}", tag=f"{nm}{g}")
                 for g in range(3)] for nm in ("hA", "hB", "hC", "hD")]
            for g in range(3):
                nc.scalar.activation(out=slA[g], in_=psts[g][:, 0, :], func=AF.Square)
                nc.scalar.activation(out=slB[g], in_=psts[g][:, 1, :], func=AF.Square)
            for g in range(3):
                nc.gpsimd.partition_all_reduce(slC[g], slA[g], channels=128,
                                               reduce_op=bass_isa.ReduceOp.add)
                nc.gpsimd.partition_all_reduce(slD[g], slB[g], channels=128,
                                               reduce_op=bass_isa.ReduceOp.add)
            ssq, sd, rv, qnA, qnB = [], [], [], [], []
            for g in range(3):      # ssq -> slot A
                t = tpool.tile([128, 256], F32, name=f"ssq{g}", tag=f"hA{g}")
                nc.vector.tensor_add(t, slC[g], slD[g])
                ssq.append(t)
            for g in range(3):      # sd -> slot B
                t = tpool.tile([128, 256], F32, name=f"sd{g}", tag=f"hB{g}")
                nc.scalar.activation(out=t, in_=ssq[g], func=AF.Sqrt,
                                     scale=1.0 / D, bias=epsb)
                sd.append(t)
            for g in range(3):      # rv -> slot C
                t = tpool.tile([128, 256], F32, name=f"rv{g}", tag=f"hC{g}")
                nc.vector.reciprocal(out=t, in_=sd[g])
                rv.append(t)
            for g, (dstA, dstB, gi, w1p) in enumerate(heads):
                a = tpool.tile([128, 256], F32, name=f"qnA{g}", tag=f"hD{g}")
                nc.vector.scalar_tensor_tensor(
                    out=a, in0=psts[g][:, 0, :], scalar=w1p[:, 0:1],
                    in1=rv[g], op0=AL.mult, op1=AL.mult)
                qnA.append(a)
                b_ = tpool.tile([128, 256], F32, name=f"qnB{g}", tag=f"hA{g}")
                nc.vector.scalar_tensor_tensor(
                    out=b_, in0=psts[g][:, 1, :], scalar=w1p[:, 1:2],
                    in1=rv[g], op0=AL.mult, op1=AL.mult)
                qnB.append(b_)
            t1s, t2s = [], []
            for g in range(3):
                t1 = tpool.tile([128, 256], F32, name=f"t1{g}", tag=f"hB{g}")
                nc.vector.tensor_mul(t1, qnA[g], csn[:, 0, :])
                t1s.append(t1)
                t2 = tpool.tile([128, 256], F32, name=f"t2{g}", tag=f"hC{g}")
                nc.gpsimd.tensor_mul(t2, qnB[g], csn[:, 2, :])
                t2s.append(t2)
            for g, (dstA, dstB, gi, w1p) in enumerate(heads):
                nc.vector.tensor_sub(dstA[:, sl], t1s[g], t2s[g])
            t3s, t4s = [], []
            for g in range(3):
                t3 = tpool.tile([128, 256], F32, name=f"t3{g}", tag=f"hB{g}")
                nc.vector.tensor_mul(t3, qnB[g], csn[:, 1, :])
                t3s.append(t3)
                t4 = tpool.tile([128, 256], F32, name=f"t4{g}", tag=f"hC{g}")
                nc.gpsimd.tensor_mul(t4, qnA[g], csn[:, 3, :])
                t4s.append(t4)
            for g, (dstA, dstB, gi, w1p) in enumerate(heads):
                nc.vector.tensor_add(dstB[:, sl], t3s[g], t4s[g])

        def epilogue(ps, c, csn):
            epilogue_heads(c, csn, epilogue_stage(ps, c))

        def load_csn(c):
            # issued from the ACT queue: a pool-reuse wait here must not
            # stall the SP DMA stream feeding the matmuls
            sl = slice(c * 256, (c + 1) * 256)
            csn = cpool.tile([128, 4, 256], F32, name="csn", tag="csn")
            nc.scalar.dma_start(out=csn[:, 0:2, :], in_=cos_d[:, :, sl])
            nc.scalar.dma_start(out=csn[:, 2:4, :], in_=sin_d[:, :, sl])
            return csn

        # merged chunks 0+1 (paced by the startup DMA stream)
        csn0 = load_csn(0)
        csn1 = load_csn(1)
        ps0 = pps.tile([128, 8, 256], F32, name="ps0", tag="ps")
        ps1 = pps.tile([128, 8, 256], F32, name="ps1", tag="ps")
        # chunk 0/1 quarters all issued in the startup stream above; only
        # chunk 2's first quarters start here
        merged_loads = {15: [(2, 0)], 17: [(2, 1)]}
        for kt in range(KT):
            for (cc, qq) in merged_loads.get(kt, ()):
                load_hq(cc, qq)
            proj_matmuls(ps0, 0, kt)
            proj_matmuls(ps1, 1, kt)
            if kt % 5 == 4:
                free_hq(0, kt // 5)
                free_hq(1, kt // 5)
        psts0 = epilogue_stage(ps0, 0)
        epilogue_heads(0, csn0, psts0)
        psts1 = epilogue_stage(ps1, 1)
        heads_pending = (1, csn1, psts1)

        def chunk_loop(ps, c):
            for kt in range(KT):
                if kt == 0:
                    load_hq(c, 3)
                    if c == 2:
                        load_hq(c, 2)
                elif c + 1 < NC256:
                    if kt == 5:
                        load_hq(c + 1, 0)
                    elif kt == 10:
                        load_hq(c + 1, 1)
                    elif kt == 15:
                        load_hq(c + 1, 2)
                proj_matmuls(ps, c, kt)
                if kt % 5 == 4:
                    free_hq(c, kt // 5)

        for c in range(2, 8):
            csn = load_csn(c)
            ps = pps.tile([128, 8, 256], F32, name="ps", tag="ps")
            chunk_loop(ps, c)
            # previous chunk's norm/rope chain drains during this chunk's
            # matmuls; this chunk's PSUM staging lands right behind it
            epilogue_heads(*heads_pending)
            psts = epilogue_stage(ps, c)
            heads_pending = (c, csn, psts)
        pP.close()

        # phase-2 PSUM pools; the heads part of chunk 7's epilogue drains on
        # DVE/ACT/Pool while the first attention rounds start
        p2b = ExitStack()
        scps = p2b.enter_context(tc.tile_pool(name="scps", bufs=1, space="PSUM"))
        avps = p2b.enter_context(tc.tile_pool(name="avps", bufs=1, space="PSUM"))
        trps = p2b.enter_context(tc.tile_pool(name="trps", bufs=2, space="PSUM"))
        wops = p2b.enter_context(tc.tile_pool(name="wops", bufs=2, space="PSUM"))

        it_ctr = [0]

        def emit_scores_exp(t, h, early=False):
            """Scores + row-max + exp for (t, h); generator with stage marks."""
            r = it_ctr[0] % 2
            it_ctr[0] += 1
            w0, ch, pad = chunk_plan(t)
            assert not early or len(ch) == 1
            wt = min(t, 8) + 1
            scs = scps.tile([128, 3, 512], F32, name="scs", tag="scs")
            nms = []
            off = 0
            for ci, nt in enumerate(ch):
                width = nt * 128
                mlist = []
                if t >= 8 and ci == 0:
                    mlist.append((0, MK_BOUND))
                dlocal = (wt - 1) - off
                if 0 <= dlocal < nt:
                    mlist.append((dlocal, MK_DIAG))
                for p_ in range(nt):
                    if off + p_ >= wt:
                        mlist.append((p_, MK_FULL))
                first = True
                for (ti, kind) in mlist:
                    nc.tensor.matmul(
                        scs[:, ci, ti * 128:(ti + 1) * 128],
                        identb, m3[:, kind, :], start=first, stop=False)
                    first = False
                kcol = (w0 + off) * 128
                for j in range(NDK):
                    nc.tensor.matmul(
                        scs[:, ci, 0:width],
                        QT[2 * h + j][:, t * 128:(t + 1) * 128],
                        KTt[j][:, kcol:kcol + width],
                        start=(first and j == 0), stop=(j == NDK - 1))
                nm = big.tile([128, 1], F32, name=f"nm{ci}_{r}", tag=f"nm{ci}_{r}")
                nc.vector.tensor_reduce(out=nm, in_=scs[:, ci, 0:width],
                                        axis=AX.X, op=AL.max, negate=True)
                nms.append(nm)
                off += nt
            yield "scores_done"
            negm = nms[0]
            if len(nms) > 1:
                negm = big.tile([128, 1], F32, name=f"negm_{r}", tag=f"negm_{r}")
                nc.vector.tensor_tensor(negm, nms[0], nms[1], op=AL.min)
                if len(nms) > 2:
                    nc.vector.tensor_tensor(negm, negm, nms[2], op=AL.min)
            yield "negm_done"
            if early:
                expf = big.tile([128, 384], F32R, name=f"expbE_{r}",
                                tag=f"expbE_{r}")
            else:
                expb = etpool.tile([128, 3, 384], F32R, name="expb", tag="expb")
                expf = expb.rearrange("p a b -> p (a b)")
            den = big.tile([128, 3], F32, name=f"den_{r}", tag=f"den_{r}")
            off = 0
            for ci, nt in enumerate(ch):
                width = nt * 128
                nc.scalar.activation(
                    out=expf[:, off * 128:off * 128 + width],
                    in_=scs[:, ci, 0:width], func=AF.Exp, bias=negm,
                    accum_out=den[:, ci:ci + 1])
                off += nt
            if debug and t == 10 and h == 0:
                nc.sync.dma_start(out=dbg["dexpb"][:, 0:1152],
                                  in_=expf.bitcast(F32))
                nc.sync.dma_start(out=dbg["dnegm"][:, :], in_=negm)
                nc.sync.dma_start(out=dbg["dden"][:, :], in_=den)
            if len(ch) == 1:
                dsv = den[:, 0:1]
            else:
                dsum = big.tile([128, 1], F32, name=f"dsum_{r}", tag=f"dsum_{r}")
                nc.vector.tensor_reduce(out=dsum, in_=den[:, 0:len(ch)],
                                        axis=AX.X, op=AL.add)
                dsv = dsum
            rden = big.tile([128, 1], F32, name=f"rden_{r}", tag=f"rden_{r}")
            nc.vector.reciprocal(out=rden, in_=dsv)
            yield (expf, rden, ch, w0)

        def run_gen(g, until):
            while True:
                v = next(g)
                if v == until or not isinstance(v, str):
                    return v

        cp_rr = [0]

        def psum_copy(dst, src):
            e = (0, 1, 0)[cp_rr[0] % 3]      # 2:1 DVE:ACT
            cp_rr[0] += 1
            if e == 0:
                nc.vector.tensor_copy(out=dst, in_=src)
            else:
                nc.scalar.copy(out=dst, in_=src)

        def emit_attn_core(state, trpool, avpool, expT, ao_dst):
            """Transposes, AV matmul, and attn-out scale into ao_dst."""
            expf, rden, ch, w0 = state
            T = sum(ch)
            expTf = expT.rearrange("p a b -> p (a b)")
            off = 0
            for ci, nt in enumerate(ch):
                width = nt * 128
                trp = trpool.tile([128, 512], F32R, name="trp", tag="trp")
                for q_ in range(nt):
                    nc.tensor.transpose(
                        trp[:, q_ * 128:(q_ + 1) * 128],
                        expf[:, (off + q_) * 128:(off + q_ + 1) * 128], ident)
                psum_copy(expTf[:, off * 128:off * 128 + width], trp[:, 0:width])
                off += nt
            avp = avpool.tile([128, D], F32, name="avp", tag="avp")
            for a in range(T):
                nc.tensor.matmul(avp, expT[:, a, :], V[:, w0 + a, :],
                                 start=(a == 0), stop=(a == T - 1))
            nc.scalar.activation(out=ao_dst, in_=avp, func=AF.Copy, scale=rden)

        # boundary rounds t=0..NEARLY-1: scores/exp/AV run while the chunk-7
        # heads chain drains; attn-out goes to big-pool slots, aoT deferred
        first = emit_scores_exp(0, 0, early=True)
        run_gen(first, "scores_done")
        epilogue_heads(*heads_pending)
        st00 = run_gen(first, None)
        emit_attn_core(st00, trps, avps, expTE, aoE[0])
        for t in range(NEARLY):
            for h in range(2):
                if t == 0 and h == 0:
                    continue
                g = emit_scores_exp(t, h, early=True)
                run_gen(g, "scores_done")
                st = run_gen(g, None)
                emit_attn_core(st, trps, avps, expTE, aoE[2 * t + h])
        pA.close()

        # ---------------- Phase 2 main + output projection ----------------
        etpool = p2b.enter_context(tc.tile_pool(name="etpool", bufs=2))
        wopool = p2b.enter_context(tc.tile_pool(name="wopool", bufs=1))
        aopool = p2b.enter_context(tc.tile_pool(name="aopool", bufs=1))
        opool = p2b.enter_context(tc.tile_pool(name="opool", bufs=2))

        woT = wopool.tile([128, NDQ, H], F32R, tag="woT")
        for dj in range(NDQ):
            nc.sync.dma_start(out=woT[:, dj:dj + 1, :], in_=wo_d[:, dj:dj + 1, :])
        aoT = [aopool.tile([128, S], F32R, name=f"aoT{j}", tag=f"aoT{j}")
               for j in range(NDQ)]

        wo_queue = []
        ost_tiles = {}

        def emit_wo_part():
            if not wo_queue:
                return
            t, part = wo_queue.pop(0)
            hcs = (0, 1, 2) if part == 0 else (3, 4)
            if part == 0:
                ost_tiles[t] = opool.tile([128, H], F32, name="ost", tag="ost")
            ost = ost_tiles[t]
            for hc in hcs:
                wop = wops.tile([128, 512], F32, name="wop", tag="wop")
                for dj in range(NDQ):
                    nc.tensor.matmul(
                        wop, aoT[dj][:, t * 128:(t + 1) * 128],
                        woT[:, dj, hc * 512:(hc + 1) * 512],
                        start=(dj == 0), stop=(dj == NDQ - 1))
                psum_copy(ost[:, hc * 512:(hc + 1) * 512], wop)
            if part == 1:
                nc.sync.dma_start(
                    out=out_d[t * 128:(t + 1) * 128, :], in_=ost_tiles.pop(t))

        def emit_aoT(t, h, ao_src):
            trp = trps.tile([128, 512], F32R, name="trpa", tag="trp")
            for j in range(2):
                nc.tensor.transpose(trp[:, j * 128:(j + 1) * 128],
                                    ao_src[:, j * 128:(j + 1) * 128], ident)
            for j in range(2):
                psum_copy(aoT[2 * h + j][:, t * 128:(t + 1) * 128],
                          trp[:, j * 128:(j + 1) * 128])

        # catch up: aoT for the early rounds, then queue their Wo parts
        for t in range(NEARLY):
            for h in range(2):
                emit_aoT(t, h, aoE[2 * t + h])
            wo_queue += [(t, 0), (t, 1)]
            if t >= 2:
                emit_wo_part()

        for t in range(NEARLY, ST):
            for h in range(2):
                g = emit_scores_exp(t, h)
                run_gen(g, "scores_done")
                emit_wo_part()
                run_gen(g, "negm_done")
                emit_wo_part()
                state = run_gen(g, None)
                ao = etpool.tile([128, D], F32R, name="ao", tag="ao")
                expT = etpool.tile([128, 9, 128], F32R, name="expT", tag="expT")
                emit_attn_core(state, trps, avps, expT, ao)
                emit_aoT(t, h, ao)
            wo_queue.append((t, 0))
            wo_queue.append((t, 1))
            if len(wo_queue) > 4:
                emit_wo_part()

        while wo_queue:
            emit_wo_part()

        p2b.close()

    nc.compile()
    return nc


_nc_cache = None


def _prep_inputs(hidden_states, attention_mask, cos, sin, Wq, Wk, Wv, Wo,
                 q_norm_w, k_norm_w):
    import ml_dtypes
    bf16 = ml_dtypes.bfloat16

    hidden_states = np.asarray(hidden_states, dtype=np.float32)
    mask = np.asarray(attention_mask, dtype=np.float32)[0, 0]      # [S, S]
    cos2 = np.asarray(cos, dtype=np.float32)[0, 0]                 # [S, D]
    sin2 = np.asarray(sin, dtype=np.float32)[0, 0]
    Wq = np.asarray(Wq, dtype=np.float32)
    Wk = np.asarray(Wk, dtype=np.float32)
    Wv = np.asarray(Wv, dtype=np.float32)
    Wo = np.asarray(Wo, dtype=np.float32)

    cosh = np.ascontiguousarray(
        cos2.T.reshape(NDK, 128, S).transpose(1, 0, 2))            # [128,2,S]
    sinh = np.ascontiguousarray(sin2.T.reshape(NDK, 128, S).transpose(1, 0, 2))

    diag = mask[0:128, 0:128]
    bound = mask[1024:1152, 0:128]
    full = mask[0:128, 1024:1152]
    m3h = np.ascontiguousarray(
        np.stack([diag, bound, full], axis=1)).astype(bf16)        # [128,3,128]

    qw1p = np.ascontiguousarray(
        (1.0 + np.asarray(q_norm_w, dtype=np.float32)).reshape(NDK, 128).T)
    kw1p = np.ascontiguousarray(
        (1.0 + np.asarray(k_norm_w, dtype=np.float32)).reshape(NDK, 128).T)
    ident = round_f32r(np.eye(128, dtype=np.float32))
    identb = np.eye(128, dtype=np.float32).astype(bf16)

    in_maps = []
    for core in range(8):
        b, g = core // 4, core % 4
        hsT = round_f32r(np.ascontiguousarray(hidden_states[b].T))  # [H, S]
        hsC = np.ascontiguousarray(
            hsT.reshape(KT, 128, NC256, 256).transpose(2, 1, 0, 3))
        wqh = round_f32r(np.ascontiguousarray(
            Wq[2 * g * D:(2 * g + 2) * D].T.reshape(KT, 128, DQ).transpose(1, 0, 2)))
        wkh = round_f32r(np.ascontiguousarray(
            Wk[g * D:(g + 1) * D].T.reshape(KT, 128, D).transpose(1, 0, 2)))
        wvh = round_f32r(np.ascontiguousarray(
            Wv[g * D:(g + 1) * D].T.reshape(KT, 128, D).transpose(1, 0, 2)))
        woh = round_f32r(np.ascontiguousarray(
            Wo[:, 2 * g * D:(2 * g + 2) * D].T.reshape(NDQ, 128, H).transpose(1, 0, 2)))
        in_maps.append({
            "hsC": hsC, "wqh": wqh, "wkh": wkh, "wvh": wvh, "woh": woh,
            "cosh": cosh, "sinh": sinh, "m3h": m3h, "ident": ident,
            "identb": identb, "qw1p": qw1p, "kw1p": kw1p,
        })
    return in_maps


def kernel(hidden_states, attention_mask, cos, sin, Wq, Wk, Wv, Wo,
           q_norm_w, k_norm_w):
    global _nc_cache
    if _nc_cache is None:
        _nc_cache = build_nc()
    nc = _nc_cache

    in_maps = _prep_inputs(hidden_states, attention_mask, cos, sin,
                           Wq, Wk, Wv, Wo, q_norm_w, k_norm_w)
    res = run_bass_kernel_spmd(nc, in_maps, core_ids=list(range(8)))
    outs = [r["out"] for r in res.results]
    final = np.zeros((B, S, H), dtype=np.float32)
    for core in range(8):
        final[core // 4] += outs[core]
    return final


# revision 3
# speedup vs baseline: 1.0589x; 1.0149x over previous
"""Gemma-style sliding-window attention block on 8 trn2 NeuronCores, v2.

Sharding: tensor-parallel over kv-head groups (4) x data-parallel over
batch (2).  Core c handles batch b = c//4 and kv-head g = c%4 (query
heads 2g, 2g+1).  Host sums the 4 partial Wo outputs per batch.

Structure:
- Phase 1 runs in 8 seq-chunks of 256 with a double-buffered 4-bank PSUM
  set; the first two chunks share a merged kt loop paced by the initial
  weight-stream DMAs.  cos/sin chunk loads are issued from the ACT queue
  so their pool-reuse waits cannot stall the SP DMA stream.
- RMS-norm partition sums and all rope multiplies run on gpsimd (SBUF
  only; the BIR verifier forbids gpsimd PSUM access).
- Attention skips fully-masked future key tiles for t<8 and initializes
  score PSUM banks with constant mask tiles (mask-first matmul).
- Attention rounds for t=0..5 run from a small early PSUM pool that
  coexists with the last projection chunk, overlapping its epilogue;
  their attn-out transposes are caught up afterwards.
- Wo projection matmuls are interleaved into the attention loop as PE
  filler work, drained from a backlog queue.
"""
import numpy as np
from contextlib import ExitStack

import concourse.bass as bass
import concourse.bacc as bacc
import concourse.mybir as mybir
import concourse.tile as tile
from concourse import bass_isa
from concourse.bass_utils import run_bass_kernel_spmd

F32 = mybir.dt.float32
F32R = mybir.dt.float32r
BF16 = mybir.dt.bfloat16
AL = mybir.AluOpType
AF = mybir.ActivationFunctionType
AX = mybir.AxisListType

B, S, H = 2, 2048, 2560
NH, NKV, D = 8, 4, 256
SW = 1024
EPS = 1e-6
ST = S // 128             # 16 q/k tiles
KT = H // 128             # 20 hidden k-tiles
NC256 = S // 256          # 8 seq chunks (phase 1)
DQ = 2 * D                # per-core query dims (2 heads)
NDQ = DQ // 128           # 4
NDK = D // 128            # 2
NEARLY = 3                # q-tiles whose attn-out/Wo work is deferred past
                          # the phase-1 SBUF pool barrier

MK_DIAG, MK_BOUND, MK_FULL = 0, 1, 2


def round_f32r(x: np.ndarray) -> np.ndarray:
    """Round fp32 to f32r (11-bit mantissa, round-to-nearest-even)."""
    b = np.ascontiguousarray(x, dtype=np.float32).view(np.uint32).astype(np.uint64)
    bias = 0x7FF + ((b >> 12) & 1)
    return ((b + bias) & 0xFFFFF000).astype(np.uint32).view(np.float32)


def chunk_plan(t):
    """(w0, [ntile per chunk], pad) for q-tile t. Chunks are 2-3 tiles so
    every matmul free dim >= 256 (full f32r rate); t=0 pads one dead tile."""
    w0 = max(0, t - 8)
    wt = min(t, 8) + 1
    plans = {1: [2], 2: [2], 3: [3], 4: [2, 2], 5: [3, 2], 6: [3, 3],
             7: [3, 2, 2], 8: [3, 3, 2], 9: [3, 3, 3]}
    ch = plans[wt]
    return w0, ch, sum(ch) - wt


def build_nc(debug=False):
    nc = bacc.Bacc("TRN2", target_bir_lowering=False, debug=False)

    hs_d = nc.dram_tensor("hsC", [NC256, 128, KT, 256], F32R, kind="ExternalInput")
    wq_d = nc.dram_tensor("wqh", [128, KT, DQ], F32R, kind="ExternalInput")
    wk_d = nc.dram_tensor("wkh", [128, KT, D], F32R, kind="ExternalInput")
    wv_d = nc.dram_tensor("wvh", [128, KT, D], F32R, kind="ExternalInput")
    wo_d = nc.dram_tensor("woh", [128, NDQ, H], F32R, kind="ExternalInput")
    cos_d = nc.dram_tensor("cosh", [128, NDK, S], F32, kind="ExternalInput")
    sin_d = nc.dram_tensor("sinh", [128, NDK, S], F32, kind="ExternalInput")
    m3_d = nc.dram_tensor("m3h", [128, 3, 128], BF16, kind="ExternalInput")
    idn_d = nc.dram_tensor("ident", [128, 128], F32R, kind="ExternalInput")
    idb_d = nc.dram_tensor("identb", [128, 128], BF16, kind="ExternalInput")
    qw_d = nc.dram_tensor("qw1p", [128, NDK], F32, kind="ExternalInput")
    w0p_d = nc.dram_tensor("w0pack", [128, 4, 256], F32R, kind="ExternalInput")
    kw_d = nc.dram_tensor("kw1p", [128, NDK], F32, kind="ExternalInput")
    out_d = nc.dram_tensor("out", [S, H], F32, kind="ExternalOutput")
    dbg = {}
    if debug:
        for nm, shp in [("dQT0", [128, S]), ("dKT0", [128, S]),
                        ("dV", [128, ST * D]), ("dexpb", [128, 1152]),
                        ("dnegm", [128, 1]), ("dden", [128, 3])]:
            dbg[nm] = nc.dram_tensor(nm, shp, F32, kind="ExternalOutput")

    with ExitStack() as top:
        tc = top.enter_context(tile.TileContext(nc))
        big = top.enter_context(tc.tile_pool(name="big", bufs=1))

        # Whole-kernel residents
        QT = [big.tile([128, S], F32R, name=f"QT{j}", tag=f"QT{j}") for j in range(NDQ)]
        KTt = [big.tile([128, S], F32R, name=f"KTt{j}", tag=f"KTt{j}") for j in range(NDK)]
        V = big.tile([128, ST, D], F32R, tag="V")
        ident = big.tile([128, 128], F32R, tag="ident")
        identb = big.tile([128, 128], BF16, tag="identb")
        m3 = big.tile([128, 3, 128], BF16, tag="m3")
        epsb = big.tile([128, 1], F32, tag="epsb")
        qw1p = big.tile([128, NDK], F32, tag="qw1p")
        kw1p = big.tile([128, NDK], F32, tag="kw1p")
        # boundary-overlap staging: attn-out and P/P^T tiles for t<NEARLY
        aoE = [big.tile([128, D], F32R, name=f"aoE{i}", tag=f"aoE{i}")
               for i in range(2 * NEARLY)]
        expTE = big.tile([128, 3, 128], F32R, name="expTE", tag="expTE")

        # ---------------- Phase 1: projections + norms + rope -------------
        pA = ExitStack()        # phase-1 SBUF pools
        wpool = pA.enter_context(tc.tile_pool(name="wpool", bufs=1))
        hpool = pA.enter_context(tc.tile_pool(name="hpool", bufs=5))
        cpool = pA.enter_context(tc.tile_pool(name="cpool", bufs=2))
        tpool = pA.enter_context(tc.tile_pool(name="tpool", bufs=1))
        spool = pA.enter_context(tc.tile_pool(name="spool", bufs=2))
        pP = ExitStack()        # phase-1 PSUM (chunks 0..6)
        pps = pP.enter_context(tc.tile_pool(name="pps", bufs=2, space="PSUM"))

        wq = wpool.tile([128, KT - 1, DQ], F32R, tag="wq")
        wk = wpool.tile([128, KT - 1, D], F32R, tag="wk")
        wv = wpool.tile([128, KT - 1, D], F32R, tag="wv")

        # hs chunk tiles in 5-kt quarters
        hquarts = {}

        def load_hq(c, q):
            ht = hpool.tile([128, 5, 256], F32R, name="hst", tag="hst")
            nc.sync.dma_start(out=ht, in_=hs_d[c, :, q * 5:(q + 1) * 5, :])
            hquarts[(c, q)] = ht

        def get_hst(c, kt):
            return hquarts[(c, kt // 5)][:, kt % 5, :]

        def free_hq(c, q):
            hquarts.pop((c, q), None)

        # staged startup: one packed first-kt weight DMA so matmuls start
        # as early as possible; kt==0 matmuls for every chunk read w0pack.
        w0pack = wpool.tile([128, 4, 256], F32R, tag="w0pack")
        w0f = w0pack.rearrange("p a b -> p (a b)")
        nc.sync.dma_start(out=w0pack, in_=w0p_d[:, :, :])
        ht00 = hpool.tile([128, 5, 256], F32R, name="hst", tag="hst")
        nc.sync.dma_start(out=ht00[:, 0:1, :], in_=hs_d[0, :, 0:1, :])
        ht10 = hpool.tile([128, 5, 256], F32R, name="hst", tag="hst")
        nc.sync.dma_start(out=ht10[:, 0:1, :], in_=hs_d[1, :, 0:1, :])
        nc.sync.dma_start(out=ht00[:, 1:5, :], in_=hs_d[0, :, 1:5, :])
        nc.sync.dma_start(out=ht10[:, 1:5, :], in_=hs_d[1, :, 1:5, :])
        hquarts[(0, 0)] = ht00
        hquarts[(1, 0)] = ht10
        nc.sync.dma_start(out=wq[:, 0:3, :], in_=wq_d[:, 1:4, :])
        nc.sync.dma_start(out=wk[:, 0:5, :], in_=wk_d[:, 1:6, :])
        nc.sync.dma_start(out=wv[:, 0:5, :], in_=wv_d[:, 1:6, :])
        nc.sync.dma_start(out=wq[:, 3:6, :], in_=wq_d[:, 4:7, :])
        load_hq(0, 1)
        load_hq(1, 1)
        nc.sync.dma_start(out=wq[:, 6:9, :], in_=wq_d[:, 7:10, :])
        nc.sync.dma_start(out=wk[:, 5:12, :], in_=wk_d[:, 6:13, :])
        nc.sync.dma_start(out=wv[:, 5:12, :], in_=wv_d[:, 6:13, :])
        load_hq(0, 2)
        nc.sync.dma_start(out=wq[:, 9:13, :], in_=wq_d[:, 10:14, :])
        load_hq(1, 2)
        nc.sync.dma_start(out=wk[:, 12:19, :], in_=wk_d[:, 13:20, :])
        nc.sync.dma_start(out=wv[:, 12:19, :], in_=wv_d[:, 13:20, :])
        load_hq(0, 3)
        nc.sync.dma_start(out=wq[:, 13:16, :], in_=wq_d[:, 14:17, :])
        load_hq(1, 3)
        nc.sync.dma_start(out=wq[:, 16:19, :], in_=wq_d[:, 17:20, :])
        nc.sync.dma_start(out=ident, in_=idn_d[:, :])
        nc.sync.dma_start(out=identb, in_=idb_d[:, :])
        nc.sync.dma_start(out=m3, in_=m3_d[:, :, :])
        nc.sync.dma_start(out=qw1p, in_=qw_d[:, :])
        nc.sync.dma_start(out=kw1p, in_=kw_d[:, :])
        nc.vector.memset(epsb, EPS)

        def proj_matmuls(ps, c, kt):
            hst = get_hst(c, kt)
            st_, sp_ = (kt == 0), (kt == KT - 1)
            if kt == 0:
                wqs = lambda j: w0f[:, j * 128:(j + 1) * 128]
                wks = lambda j: w0f[:, 512 + j * 128:512 + (j + 1) * 128]
                wvs = w0f[:, 768:1024]
            else:
                wqs = lambda j: wq[:, kt - 1, j * 128:(j + 1) * 128]
                wks = lambda j: wk[:, kt - 1, j * 128:(j + 1) * 128]
                wvs = wv[:, kt - 1, :]
            for j in range(NDQ):
                nc.tensor.matmul(ps[:, j, :], wqs(j),
                                 hst, start=(st_ and j % 2 == 0), stop=sp_)
            for j in range(NDK):
                nc.tensor.matmul(ps[:, 4 + j, :], wks(j),
                                 hst, start=(st_ and j == 0), stop=sp_)
            for i in range(2):
                nc.tensor.matmul(ps[:, 6 + i, :], hst[:, i * 128:(i + 1) * 128],
                                 wvs, start=(st_ and i == 0), stop=sp_)

        def epilogue_stage(ps, c):
            """All ps readers up front so the PSUM set releases quickly."""
            msqs = []
            for i in range(2):
                msq = spool.tile([128, 1], F32, name=f"msq{i}", tag=f"msq{i}")
                vsq = tpool.tile([128, D], F32, name="vsq", tag="hA0")
                nc.scalar.activation(out=vsq, in_=ps[:, 6 + i, :],
                                     func=AF.Square, accum_out=msq)
                msqs.append(msq)
            psts = []
            for gi in range(3):
                pst = tpool.tile([128, 2, 256], F32, name=f"pst{gi}",
                                 tag=f"pst{gi}")
                nc.scalar.copy(out=pst, in_=ps[:, 2 * gi:2 * gi + 2, :])
                psts.append(pst)
            for i in range(2):
                sdv = spool.tile([128, 1], F32, tag="sdv")
                nc.scalar.activation(out=sdv, in_=msqs[i], func=AF.Sqrt,
                                     scale=1.0 / D, bias=epsb)
                rvv = spool.tile([128, 1], F32, tag="rvv")
                nc.vector.reciprocal(out=rvv, in_=sdv)
                nc.scalar.activation(out=V[:, 2 * c + i, :], in_=ps[:, 6 + i, :],
                                     func=AF.Copy, scale=rvv)
            return psts

        def epilogue_heads(c, csn, psts):
            """Norm/rope chain from the SBUF staging tiles.

            Breadth-first emission: same-stage ops across the three head
            groups go out together so the in-order engine queues execute
            the groups concurrently instead of serializing the chains.
            """
            sl = slice(c * 256, (c + 1) * 256)
            heads = [(QT[0], QT[1], 0, qw1p), (QT[2], QT[3], 1, qw1p),
                     (KTt[0], KTt[1], 2, kw1p)]
            slA, slB, slC, slD = [
                [tpool.tile([128, 256], F32, name=f"{nm}{g}", tag=f"{nm}{g}")
                 for g in range(3)] for nm in ("hA", "hB", "hC", "hD")]
            for g in range(3):
                nc.scalar.activation(out=slA[g], in_=psts[g][:, 0, :], func=AF.Square)
                nc.scalar.activation(out=slB[g], in_=psts[g][:, 1, :], func=AF.Square)
            for g in range(3):
                nc.gpsimd.partition_all_reduce(slC[g], slA[g], channels=128,
                                               reduce_op=bass_isa.ReduceOp.add)
                nc.gpsimd.partition_all_reduce(slD[g], slB[g], channels=128,
                                               reduce_op=bass_isa.ReduceOp.add)
            ssq, sd, rv, qnA, qnB = [], [], [], [], []
            for g in range(3):      # ssq -> slot A
                t = tpool.tile([128, 256], F32, name=f"ssq{g}", tag=f"hA{g}")
                nc.vector.tensor_add(t, slC[g], slD[g])
                ssq.append(t)
            for g in range(3):      # sd -> slot B
                t = tpool.tile([128, 256], F32, name=f"sd{g}", tag=f"hB{g}")
                nc.scalar.activation(out=t, in_=ssq[g], func=AF.Sqrt,
                                     scale=1.0 / D, bias=epsb)
                sd.append(t)
            for g in range(3):      # rv -> slot C
                t = tpool.tile([128, 256], F32, name=f"rv{g}", tag=f"hC{g}")
                nc.vector.reciprocal(out=t, in_=sd[g])
                rv.append(t)
            for g, (dstA, dstB, gi, w1p) in enumerate(heads):
                a = tpool.tile([128, 256], F32, name=f"qnA{g}", tag=f"hD{g}")
                nc.vector.scalar_tensor_tensor(
                    out=a, in0=psts[g][:, 0, :], scalar=w1p[:, 0:1],
                    in1=rv[g], op0=AL.mult, op1=AL.mult)
                qnA.append(a)
                b_ = tpool.tile([128, 256], F32, name=f"qnB{g}", tag=f"hA{g}")
                nc.vector.scalar_tensor_tensor(
                    out=b_, in0=psts[g][:, 1, :], scalar=w1p[:, 1:2],
                    in1=rv[g], op0=AL.mult, op1=AL.mult)
                qnB.append(b_)
            t1s, t2s = [], []
            for g in range(3):
                t1 = tpool.tile([128, 256], F32, name=f"t1{g}", tag=f"hB{g}")
                nc.vector.tensor_mul(t1, qnA[g], csn[:, 0, :])
                t1s.append(t1)
                t2 = tpool.tile([128, 256], F32, name=f"t2{g}", tag=f"hC{g}")
                nc.gpsimd.tensor_mul(t2, qnB[g], csn[:, 2, :])
                t2s.append(t2)
            for g, (dstA, dstB, gi, w1p) in enumerate(heads):
                nc.vector.tensor_sub(dstA[:, sl], t1s[g], t2s[g])
            t3s, t4s = [], []
            for g in range(3):
                t3 = tpool.tile([128, 256], F32, name=f"t3{g}", tag=f"hB{g}")
                nc.vector.tensor_mul(t3, qnB[g], csn[:, 1, :])
                t3s.append(t3)
                t4 = tpool.tile([128, 256], F32, name=f"t4{g}", tag=f"hC{g}")
                nc.gpsimd.tensor_mul(t4, qnA[g], csn[:, 3, :])
                t4s.append(t4)
            for g, (dstA, dstB, gi, w1p) in enumerate(heads):
                nc.vector.tensor_add(dstB[:, sl], t3s[g], t4s[g])

        def epilogue(ps, c, csn):
            epilogue_heads(c, csn, epilogue_stage(ps, c))

        def load_csn(c):
            # issued from the ACT queue: a pool-reuse wait here must not
            # stall the SP DMA stream feeding the matmuls
            sl = slice(c * 256, (c + 1) * 256)
            csn = cpool.tile([128, 4, 256], F32, name="csn", tag="csn")
            nc.scalar.dma_start(out=csn[:, 0:2, :], in_=cos_d[:, :, sl])
            nc.scalar.dma_start(out=csn[:, 2:4, :], in_=sin_d[:, :, sl])
            return csn

        # merged chunks 0+1 (paced by the startup DMA stream)
        csn0 = load_csn(0)
        csn1 = load_csn(1)
        ps0 = pps.tile([128, 8, 256], F32, name="ps0", tag="ps")
        ps1 = pps.tile([128, 8, 256], F32, name="ps1", tag="ps")
        # chunk 0/1 quarters all issued in the startup stream above; only
        # chunk 2's first quarters start here
        merged_loads = {15: [(2, 0)], 17: [(2, 1)]}
        for kt in range(KT):
            for (cc, qq) in merged_loads.get(kt, ()):
                load_hq(cc, qq)
            proj_matmuls(ps0, 0, kt)
            proj_matmuls(ps1, 1, kt)
            if kt % 5 == 4:
                free_hq(0, kt // 5)
                free_hq(1, kt // 5)
        psts0 = epilogue_stage(ps0, 0)
        epilogue_heads(0, csn0, psts0)
        psts1 = epilogue_stage(ps1, 1)
        heads_pending = (1, csn1, psts1)

        def chunk_loop(ps, c):
            for kt in range(KT):
                if kt == 0:
                    load_hq(c, 3)
                    if c == 2:
                        load_hq(c, 2)
                elif c + 1 < NC256:
                    if kt == 5:
                        load_hq(c + 1, 0)
                    elif kt == 10:
                        load_hq(c + 1, 1)
                    elif kt == 15:
                        load_hq(c + 1, 2)
                proj_matmuls(ps, c, kt)
                if kt % 5 == 4:
                    free_hq(c, kt // 5)

        for c in range(2, 8):
            csn = load_csn(c)
            ps = pps.tile([128, 8, 256], F32, name="ps", tag="ps")
            chunk_loop(ps, c)
            # previous chunk's norm/rope chain drains during this chunk's
            # matmuls; this chunk's PSUM staging lands right behind it
            epilogue_heads(*heads_pending)
            psts = epilogue_stage(ps, c)
            heads_pending = (c, csn, psts)
        pP.close()

        # phase-2 PSUM pools; the heads part of chunk 7's epilogue drains on
        # DVE/ACT/Pool while the first attention rounds start
        p2b = ExitStack()
        scps = p2b.enter_context(tc.tile_pool(name="scps", bufs=1, space="PSUM"))
        avps = p2b.enter_context(tc.tile_pool(name="avps", bufs=1, space="PSUM"))
        trps = p2b.enter_context(tc.tile_pool(name="trps", bufs=2, space="PSUM"))
        wops = p2b.enter_context(tc.tile_pool(name="wops", bufs=2, space="PSUM"))

        it_ctr = [0]

        def emit_scores_exp(t, h, early=False):
            """Scores + row-max + exp for (t, h); generator with stage marks.

            Early (single-chunk) rounds take their score bank from the
            still-idle wops pool so two rounds can pipeline before the main
            pools open."""
            r = it_ctr[0] % 2
            it_ctr[0] += 1
            w0, ch, pad = chunk_plan(t)
            assert not early or len(ch) == 1
            wt = min(t, 8) + 1
            if early:
                sc2 = wops.tile([128, 512], F32, name="scsE", tag="wop")
                chunk_ap = lambda ci: sc2
            else:
                scs = scps.tile([128, 3, 512], F32, name="scs", tag="scs")
                chunk_ap = lambda ci: scs[:, ci]
            nms = []
            off = 0
            for ci, nt in enumerate(ch):
                width = nt * 128
                ca = chunk_ap(ci)
                mlist = []
                if t >= 8 and ci == 0:
                    mlist.append((0, MK_BOUND))
                dlocal = (wt - 1) - off
                if 0 <= dlocal < nt:
                    mlist.append((dlocal, MK_DIAG))
                for p_ in range(nt):
                    if off + p_ >= wt:
                        mlist.append((p_, MK_FULL))
                first = True
                for (ti, kind) in mlist:
                    nc.tensor.matmul(
                        ca[:, ti * 128:(ti + 1) * 128],
                        identb, m3[:, kind, :], start=first, stop=False)
                    first = False
                kcol = (w0 + off) * 128
                for j in range(NDK):
                    nc.tensor.matmul(
                        ca[:, 0:width],
                        QT[2 * h + j][:, t * 128:(t + 1) * 128],
                        KTt[j][:, kcol:kcol + width],
                        start=(first and j == 0), stop=(j == NDK - 1))
                off += nt
                # row-max: batch the leading equal-width chunk pair
                if not early and ci == 1 and ch[0] == ch[1]:
                    nm = big.tile([128, 1], F32, name=f"nm0_{r}", tag=f"nm0_{r}")
                    nc.vector.tensor_reduce(out=nm, in_=scs[:, 0:2, 0:width],
                                            axis=AX.XY, op=AL.max, negate=True)
                    nms = [nm]
                elif not (not early and ci == 0 and len(ch) > 1 and ch[0] == ch[1]):
                    nm = big.tile([128, 1], F32, name=f"nm{ci}_{r}", tag=f"nm{ci}_{r}")
                    nc.vector.tensor_reduce(out=nm, in_=ca[:, 0:width],
                                            axis=AX.X, op=AL.max, negate=True)
                    nms.append(nm)
            yield "scores_done"
            negm = nms[0]
            if len(nms) > 1:
                negm = big.tile([128, 1], F32, name=f"negm_{r}", tag=f"negm_{r}")
                nc.vector.tensor_tensor(negm, nms[0], nms[1], op=AL.min)
                if len(nms) > 2:
                    nc.vector.tensor_tensor(negm, negm, nms[2], op=AL.min)
            yield "negm_done"
            if early:
                expf = big.tile([128, 384], F32R, name=f"expbE_{r}",
                                tag=f"expbE_{r}")
            else:
                expb = etpool.tile([128, 3, 384], F32R, name="expb", tag="expb")
                expf = expb.rearrange("p a b -> p (a b)")
            den = big.tile([128, 3], F32, name=f"den_{r}", tag=f"den_{r}")
            off = 0
            for ci, nt in enumerate(ch):
                width = nt * 128
                nc.scalar.activation(
                    out=expf[:, off * 128:off * 128 + width],
                    in_=chunk_ap(ci)[:, 0:width], func=AF.Exp, bias=negm,
                    accum_out=den[:, ci:ci + 1])
                off += nt
            if debug and t == 10 and h == 0:
                nc.sync.dma_start(out=dbg["dexpb"][:, 0:1152],
                                  in_=expf.bitcast(F32))
                nc.sync.dma_start(out=dbg["dnegm"][:, :], in_=negm)
                nc.sync.dma_start(out=dbg["dden"][:, :], in_=den)
            if len(ch) == 1:
                dsv = den[:, 0:1]
            else:
                dsum = big.tile([128, 1], F32, name=f"dsum_{r}", tag=f"dsum_{r}")
                nc.vector.tensor_reduce(out=dsum, in_=den[:, 0:len(ch)],
                                        axis=AX.X, op=AL.add)
                dsv = dsum
            rden = big.tile([128, 1], F32, name=f"rden_{r}", tag=f"rden_{r}")
            nc.vector.reciprocal(out=rden, in_=dsv)
            yield (expf, rden, ch, w0)

        def run_gen(g, until):
            while True:
                v = next(g)
                if v == until or not isinstance(v, str):
                    return v

        cp_rr = [0]

        def psum_copy(dst, src):
            e = (0, 1, 0)[cp_rr[0] % 3]      # 2:1 DVE:ACT
            cp_rr[0] += 1
            if e == 0:
                nc.vector.tensor_copy(out=dst, in_=src)
            else:
                nc.scalar.copy(out=dst, in_=src)

        def emit_attn_core(state, trpool, avpool, expT, ao_dst):
            """Transposes, AV matmul, and attn-out scale into ao_dst."""
            expf, rden, ch, w0 = state
            T = sum(ch)
            expTf = expT.rearrange("p a b -> p (a b)")
            off = 0
            for ci, nt in enumerate(ch):
                width = nt * 128
                trp = trpool.tile([128, 512], F32R, name="trp", tag="trp")
                for q_ in range(nt):
                    nc.tensor.transpose(
                        trp[:, q_ * 128:(q_ + 1) * 128],
                        expf[:, (off + q_) * 128:(off + q_ + 1) * 128], ident)
                psum_copy(expTf[:, off * 128:off * 128 + width], trp[:, 0:width])
                off += nt
            avp = avpool.tile([128, D], F32, name="avp", tag="avp")
            for a in range(T):
                nc.tensor.matmul(avp, expT[:, a, :], V[:, w0 + a, :],
                                 start=(a == 0), stop=(a == T - 1))
            nc.scalar.activation(out=ao_dst, in_=avp, func=AF.Copy, scale=rden)

        # boundary rounds t=0..NEARLY-1: scores/exp/AV run while the chunk-7
        # heads chain drains; attn-out goes to big-pool slots, aoT deferred
        first = emit_scores_exp(0, 0, early=True)
        run_gen(first, "scores_done")
        epilogue_heads(*heads_pending)
        st00 = run_gen(first, None)
        emit_attn_core(st00, trps, avps, expTE, aoE[0])
        for t in range(NEARLY):
            for h in range(2):
                if t == 0 and h == 0:
                    continue
                g = emit_scores_exp(t, h, early=True)
                run_gen(g, "scores_done")
                st = run_gen(g, None)
                emit_attn_core(st, trps, avps, expTE, aoE[2 * t + h])
        pA.close()

        # ---------------- Phase 2 main + output projection ----------------
        etpool = p2b.enter_context(tc.tile_pool(name="etpool", bufs=2))
        wopool = p2b.enter_context(tc.tile_pool(name="wopool", bufs=1))
        aopool = p2b.enter_context(tc.tile_pool(name="aopool", bufs=1))
        opool = p2b.enter_context(tc.tile_pool(name="opool", bufs=2))

        woT = wopool.tile([128, NDQ, H], F32R, tag="woT")
        for dj in range(NDQ):
            nc.sync.dma_start(out=woT[:, dj:dj + 1, :], in_=wo_d[:, dj:dj + 1, :])
        aoT = [aopool.tile([128, S], F32R, name=f"aoT{j}", tag=f"aoT{j}")
               for j in range(NDQ)]

        wo_queue = []
        ost_tiles = {}

        def emit_wo_part(min_backlog=0):
            """Emit one Wo output column chunk (4 matmuls + copy) from the
            backlog; fine granules let PE fill sub-us dependency holes."""
            if len(wo_queue) <= min_backlog:
                return
            t, hc = wo_queue.pop(0)
            if hc == 0:
                ost_tiles[t] = opool.tile([128, H], F32, name="ost", tag="ost")
            ost = ost_tiles[t]
            wop = wops.tile([128, 512], F32, name="wop", tag="wop")
            for dj in range(NDQ):
                nc.tensor.matmul(
                    wop, aoT[dj][:, t * 128:(t + 1) * 128],
                    woT[:, dj, hc * 512:(hc + 1) * 512],
                    start=(dj == 0), stop=(dj == NDQ - 1))
            psum_copy(ost[:, hc * 512:(hc + 1) * 512], wop)
            if hc == 2:
                nc.sync.dma_start(
                    out=out_d[t * 128:(t + 1) * 128, 0:1536],
                    in_=ost[:, 0:1536])
            elif hc == 4:
                nc.sync.dma_start(
                    out=out_d[t * 128:(t + 1) * 128, 1536:H],
                    in_=ost_tiles.pop(t)[:, 1536:H])

        def emit_aoT(t, h, ao_src):
            trp = trps.tile([128, 512], F32R, name="trpa", tag="trp")
            for j in range(2):
                nc.tensor.transpose(trp[:, j * 128:(j + 1) * 128],
                                    ao_src[:, j * 128:(j + 1) * 128], ident)
            for j in range(2):
                psum_copy(aoT[2 * h + j][:, t * 128:(t + 1) * 128],
                          trp[:, j * 128:(j + 1) * 128])

        # catch up: aoT for the early rounds, then queue their Wo parts
        for t in range(NEARLY):
            for h in range(2):
                emit_aoT(t, h, aoE[2 * t + h])
            wo_queue += [(t, hc) for hc in range(5)]
            if t >= 2:
                emit_wo_part()
                emit_wo_part()

        for t in range(NEARLY, ST):
            for h in range(2):
                g = emit_scores_exp(t, h)
                run_gen(g, "scores_done")
                emit_wo_part()
                emit_wo_part(min_backlog=4)
                run_gen(g, "negm_done")
                emit_wo_part(min_backlog=2)
                state = run_gen(g, None)
                ao = etpool.tile([128, D], F32R, name="ao", tag="ao")
                expT = etpool.tile([128, 9, 128], F32R, name="expT", tag="expT")
                emit_attn_core(state, trps, avps, expT, ao)
                emit_wo_part(min_backlog=6)
                emit_aoT(t, h, ao)
            wo_queue += [(t, hc) for hc in range(5)]
            emit_wo_part(min_backlog=8)

        while wo_queue:
            emit_wo_part()

        p2b.close()

    nc.compile()
    return nc


_nc_cache = None


def _prep_inputs(hidden_states, attention_mask, cos, sin, Wq, Wk, Wv, Wo,
                 q_norm_w, k_norm_w):
    import ml_dtypes
    bf16 = ml_dtypes.bfloat16

    hidden_states = np.asarray(hidden_states, dtype=np.float32)
    mask = np.asarray(attention_mask, dtype=np.float32)[0, 0]      # [S, S]
    cos2 = np.asarray(cos, dtype=np.float32)[0, 0]                 # [S, D]
    sin2 = np.asarray(sin, dtype=np.float32)[0, 0]
    Wq = np.asarray(Wq, dtype=np.float32)
    Wk = np.asarray(Wk, dtype=np.float32)
    Wv = np.asarray(Wv, dtype=np.float32)
    Wo = np.asarray(Wo, dtype=np.float32)

    cosh = np.ascontiguousarray(
        cos2.T.reshape(NDK, 128, S).transpose(1, 0, 2))            # [128,2,S]
    sinh = np.ascontiguousarray(sin2.T.reshape(NDK, 128, S).transpose(1, 0, 2))

    diag = mask[0:128, 0:128]
    bound = mask[1024:1152, 0:128]
    full = mask[0:128, 1024:1152]
    m3h = np.ascontiguousarray(
        np.stack([diag, bound, full], axis=1)).astype(bf16)        # [128,3,128]

    qw1p = np.ascontiguousarray(
        (1.0 + np.asarray(q_norm_w, dtype=np.float32)).reshape(NDK, 128).T)
    kw1p = np.ascontiguousarray(
        (1.0 + np.asarray(k_norm_w, dtype=np.float32)).reshape(NDK, 128).T)
    ident = round_f32r(np.eye(128, dtype=np.float32))
    identb = np.eye(128, dtype=np.float32).astype(bf16)

    in_maps = []
    for core in range(8):
        b, g = core // 4, core % 4
        hsT = round_f32r(np.ascontiguousarray(hidden_states[b].T))  # [H, S]
        hsC = np.ascontiguousarray(
            hsT.reshape(KT, 128, NC256, 256).transpose(2, 1, 0, 3))
        wqh = round_f32r(np.ascontiguousarray(
            Wq[2 * g * D:(2 * g + 2) * D].T.reshape(KT, 128, DQ).transpose(1, 0, 2)))
        wkh = round_f32r(np.ascontiguousarray(
            Wk[g * D:(g + 1) * D].T.reshape(KT, 128, D).transpose(1, 0, 2)))
        wvh = round_f32r(np.ascontiguousarray(
            Wv[g * D:(g + 1) * D].T.reshape(KT, 128, D).transpose(1, 0, 2)))
        woh = round_f32r(np.ascontiguousarray(
            Wo[:, 2 * g * D:(2 * g + 2) * D].T.reshape(NDQ, 128, H).transpose(1, 0, 2)))
        w0pack = np.ascontiguousarray(
            np.concatenate([wqh[:, 0, :], wkh[:, 0, :], wvh[:, 0, :]],
                           axis=1).reshape(128, 4, 256))
        in_maps.append({
            "hsC": hsC, "wqh": wqh, "wkh": wkh, "wvh": wvh, "woh": woh,
            "cosh": cosh, "sinh": sinh, "m3h": m3h, "ident": ident,
            "identb": identb, "qw1p": qw1p, "kw1p": kw1p, "w0pack": w0pack,
        })
    return in_maps


def kernel(hidden_states, attention_mask, cos, sin, Wq, Wk, Wv, Wo,
           q_norm_w, k_norm_w):
    global _nc_cache
    if _nc_cache is None:
        _nc_cache = build_nc()
    nc = _nc_cache

    in_maps = _prep_inputs(hidden_states, attention_mask, cos, sin,
                           Wq, Wk, Wv, Wo, q_norm_w, k_norm_w)
    res = run_bass_kernel_spmd(nc, in_maps, core_ids=list(range(8)))
    outs = [r["out"] for r in res.results]
    final = np.zeros((B, S, H), dtype=np.float32)
    for core in range(8):
        final[core // 4] += outs[core]
    return final


# revision 4
# speedup vs baseline: 1.0601x; 1.0011x over previous
"""Gemma-style sliding-window attention block on 8 trn2 NeuronCores, v2.

Sharding: tensor-parallel over kv-head groups (4) x data-parallel over
batch (2).  Core c handles batch b = c//4 and kv-head g = c%4 (query
heads 2g, 2g+1).  Host sums the 4 partial Wo outputs per batch.

Structure:
- Phase 1 runs in 8 seq-chunks of 256 with a double-buffered 4-bank PSUM
  set; the first two chunks share a merged kt loop paced by the initial
  weight-stream DMAs.  cos/sin chunk loads are issued from the ACT queue
  so their pool-reuse waits cannot stall the SP DMA stream.
- RMS-norm partition sums and all rope multiplies run on gpsimd (SBUF
  only; the BIR verifier forbids gpsimd PSUM access).
- Attention skips fully-masked future key tiles for t<8 and initializes
  score PSUM banks with constant mask tiles (mask-first matmul).
- Attention rounds for t=0..5 run from a small early PSUM pool that
  coexists with the last projection chunk, overlapping its epilogue;
  their attn-out transposes are caught up afterwards.
- Wo projection matmuls are interleaved into the attention loop as PE
  filler work, drained from a backlog queue.
"""
import numpy as np
from contextlib import ExitStack

import concourse.bass as bass
import concourse.bacc as bacc
import concourse.mybir as mybir
import concourse.tile as tile
from concourse import bass_isa
from concourse.bass_utils import run_bass_kernel_spmd

F32 = mybir.dt.float32
F32R = mybir.dt.float32r
BF16 = mybir.dt.bfloat16
AL = mybir.AluOpType
AF = mybir.ActivationFunctionType
AX = mybir.AxisListType

B, S, H = 2, 2048, 2560
NH, NKV, D = 8, 4, 256
SW = 1024
EPS = 1e-6
ST = S // 128             # 16 q/k tiles
KT = H // 128             # 20 hidden k-tiles
NC256 = S // 256          # 8 seq chunks (phase 1)
DQ = 2 * D                # per-core query dims (2 heads)
NDQ = DQ // 128           # 4
NDK = D // 128            # 2
NEARLY = 3                # q-tiles whose attn-out/Wo work is deferred past
                          # the phase-1 SBUF pool barrier

MK_DIAG, MK_BOUND, MK_FULL = 0, 1, 2


def round_f32r(x: np.ndarray) -> np.ndarray:
    """Round fp32 to f32r (11-bit mantissa, round-to-nearest-even)."""
    b = np.ascontiguousarray(x, dtype=np.float32).view(np.uint32).astype(np.uint64)
    bias = 0x7FF + ((b >> 12) & 1)
    return ((b + bias) & 0xFFFFF000).astype(np.uint32).view(np.float32)


def chunk_plan(t):
    """(w0, [ntile per chunk], pad) for q-tile t. Chunks are 2-3 tiles so
    every matmul free dim >= 256 (full f32r rate); t=0 pads one dead tile."""
    w0 = max(0, t - 8)
    wt = min(t, 8) + 1
    plans = {1: [2], 2: [2], 3: [3], 4: [2, 2], 5: [3, 2], 6: [3, 3],
             7: [3, 2, 2], 8: [3, 3, 2], 9: [3, 3, 3]}
    ch = plans[wt]
    return w0, ch, sum(ch) - wt


def build_nc(debug=False):
    nc = bacc.Bacc("TRN2", target_bir_lowering=False, debug=False)

    hs_d = nc.dram_tensor("hsC", [NC256, 128, KT, 256], F32R, kind="ExternalInput")
    wq_d = nc.dram_tensor("wqh", [128, KT, DQ], F32R, kind="ExternalInput")
    wk_d = nc.dram_tensor("wkh", [128, KT, D], F32R, kind="ExternalInput")
    wv_d = nc.dram_tensor("wvh", [128, KT, D], F32R, kind="ExternalInput")
    wo_d = nc.dram_tensor("woh", [128, NDQ, H], F32R, kind="ExternalInput")
    cos_d = nc.dram_tensor("cosh", [128, NDK, S], F32, kind="ExternalInput")
    sin_d = nc.dram_tensor("sinh", [128, NDK, S], F32, kind="ExternalInput")
    m3_d = nc.dram_tensor("m3h", [128, 3, 128], BF16, kind="ExternalInput")
    idn_d = nc.dram_tensor("ident", [128, 128], F32R, kind="ExternalInput")
    idb_d = nc.dram_tensor("identb", [128, 128], BF16, kind="ExternalInput")
    qw_d = nc.dram_tensor("qw1p", [128, NDK], F32, kind="ExternalInput")
    w0p_d = nc.dram_tensor("w0pack", [128, 4, 256], F32R, kind="ExternalInput")
    kw_d = nc.dram_tensor("kw1p", [128, NDK], F32, kind="ExternalInput")
    out_d = nc.dram_tensor("out", [S, H], F32, kind="ExternalOutput")
    dbg = {}
    if debug:
        for nm, shp in [("dQT0", [128, S]), ("dKT0", [128, S]),
                        ("dV", [128, ST * D]), ("dexpb", [128, 1152]),
                        ("dnegm", [128, 1]), ("dden", [128, 3])]:
            dbg[nm] = nc.dram_tensor(nm, shp, F32, kind="ExternalOutput")

    with ExitStack() as top:
        tc = top.enter_context(tile.TileContext(nc))
        big = top.enter_context(tc.tile_pool(name="big", bufs=1))

        # Whole-kernel residents
        QT = [big.tile([128, S], F32R, name=f"QT{j}", tag=f"QT{j}") for j in range(NDQ)]
        KTt = [big.tile([128, S], F32R, name=f"KTt{j}", tag=f"KTt{j}") for j in range(NDK)]
        V = big.tile([128, ST, D], F32R, tag="V")
        ident = big.tile([128, 128], F32R, tag="ident")
        identb = big.tile([128, 128], BF16, tag="identb")
        m3 = big.tile([128, 3, 128], BF16, tag="m3")
        epsb = big.tile([128, 1], F32, tag="epsb")
        qw1p = big.tile([128, NDK], F32, tag="qw1p")
        kw1p = big.tile([128, NDK], F32, tag="kw1p")
        # boundary-overlap staging: attn-out and P/P^T tiles for t<NEARLY
        aoE = [big.tile([128, D], F32R, name=f"aoE{i}", tag=f"aoE{i}")
               for i in range(2 * NEARLY)]
        expTE = big.tile([128, 3, 128], F32R, name="expTE", tag="expTE")

        # ---------------- Phase 1: projections + norms + rope -------------
        pA = ExitStack()        # phase-1 SBUF pools
        wpool = pA.enter_context(tc.tile_pool(name="wpool", bufs=1))
        hpool = pA.enter_context(tc.tile_pool(name="hpool", bufs=5))
        cpool = pA.enter_context(tc.tile_pool(name="cpool", bufs=2))
        tpool = pA.enter_context(tc.tile_pool(name="tpool", bufs=1))
        spool = pA.enter_context(tc.tile_pool(name="spool", bufs=2))
        pP = ExitStack()        # phase-1 PSUM (chunks 0..6)
        pps = pP.enter_context(tc.tile_pool(name="pps", bufs=2, space="PSUM"))

        wq = wpool.tile([128, KT - 1, DQ], F32R, tag="wq")
        wk = wpool.tile([128, KT - 1, D], F32R, tag="wk")
        wv = wpool.tile([128, KT - 1, D], F32R, tag="wv")

        # hs chunk tiles in 5-kt quarters
        hquarts = {}

        def load_hq(c, q):
            ht = hpool.tile([128, 5, 256], F32R, name="hst", tag="hst")
            nc.sync.dma_start(out=ht, in_=hs_d[c, :, q * 5:(q + 1) * 5, :])
            hquarts[(c, q)] = ht

        def get_hst(c, kt):
            return hquarts[(c, kt // 5)][:, kt % 5, :]

        def free_hq(c, q):
            hquarts.pop((c, q), None)

        # staged startup: one packed first-kt weight DMA so matmuls start
        # as early as possible; kt==0 matmuls for every chunk read w0pack.
        w0pack = wpool.tile([128, 4, 256], F32R, tag="w0pack")
        w0f = w0pack.rearrange("p a b -> p (a b)")
        nc.sync.dma_start(out=w0pack, in_=w0p_d[:, :, :])
        ht00 = hpool.tile([128, 5, 256], F32R, name="hst", tag="hst")
        nc.sync.dma_start(out=ht00[:, 0:1, :], in_=hs_d[0, :, 0:1, :])
        ht10 = hpool.tile([128, 5, 256], F32R, name="hst", tag="hst")
        nc.sync.dma_start(out=ht10[:, 0:1, :], in_=hs_d[1, :, 0:1, :])
        nc.sync.dma_start(out=ht00[:, 1:5, :], in_=hs_d[0, :, 1:5, :])
        nc.sync.dma_start(out=ht10[:, 1:5, :], in_=hs_d[1, :, 1:5, :])
        hquarts[(0, 0)] = ht00
        hquarts[(1, 0)] = ht10
        nc.sync.dma_start(out=wq[:, 0:3, :], in_=wq_d[:, 1:4, :])
        nc.sync.dma_start(out=wk[:, 0:5, :], in_=wk_d[:, 1:6, :])
        nc.sync.dma_start(out=wv[:, 0:5, :], in_=wv_d[:, 1:6, :])
        nc.sync.dma_start(out=wq[:, 3:6, :], in_=wq_d[:, 4:7, :])
        load_hq(0, 1)
        load_hq(1, 1)
        nc.sync.dma_start(out=wq[:, 6:9, :], in_=wq_d[:, 7:10, :])
        nc.sync.dma_start(out=wk[:, 5:12, :], in_=wk_d[:, 6:13, :])
        nc.sync.dma_start(out=wv[:, 5:12, :], in_=wv_d[:, 6:13, :])
        load_hq(0, 2)
        nc.sync.dma_start(out=wq[:, 9:13, :], in_=wq_d[:, 10:14, :])
        load_hq(1, 2)
        nc.sync.dma_start(out=wk[:, 12:19, :], in_=wk_d[:, 13:20, :])
        nc.sync.dma_start(out=wv[:, 12:19, :], in_=wv_d[:, 13:20, :])
        load_hq(0, 3)
        nc.sync.dma_start(out=wq[:, 13:16, :], in_=wq_d[:, 14:17, :])
        load_hq(1, 3)
        nc.sync.dma_start(out=wq[:, 16:19, :], in_=wq_d[:, 17:20, :])
        nc.sync.dma_start(out=ident, in_=idn_d[:, :])
        nc.sync.dma_start(out=identb, in_=idb_d[:, :])
        nc.sync.dma_start(out=m3, in_=m3_d[:, :, :])
        nc.sync.dma_start(out=qw1p, in_=qw_d[:, :])
        nc.sync.dma_start(out=kw1p, in_=kw_d[:, :])
        nc.vector.memset(epsb, EPS)
        actwarm = big.tile([128, 1], F32, tag="actwarm")
        for fn in (AF.Square, AF.Sqrt, AF.Exp, AF.Copy):
            nc.scalar.activation(out=actwarm, in_=epsb, func=fn)

        def proj_matmuls(ps, c, kt):
            hst = get_hst(c, kt)
            st_, sp_ = (kt == 0), (kt == KT - 1)
            if kt == 0:
                wqs = lambda j: w0f[:, j * 128:(j + 1) * 128]
                wks = lambda j: w0f[:, 512 + j * 128:512 + (j + 1) * 128]
                wvs = w0f[:, 768:1024]
            else:
                wqs = lambda j: wq[:, kt - 1, j * 128:(j + 1) * 128]
                wks = lambda j: wk[:, kt - 1, j * 128:(j + 1) * 128]
                wvs = wv[:, kt - 1, :]
            for j in range(NDQ):
                nc.tensor.matmul(ps[:, j, :], wqs(j),
                                 hst, start=(st_ and j % 2 == 0), stop=sp_)
            for j in range(NDK):
                nc.tensor.matmul(ps[:, 4 + j, :], wks(j),
                                 hst, start=(st_ and j == 0), stop=sp_)
            for i in range(2):
                nc.tensor.matmul(ps[:, 6 + i, :], hst[:, i * 128:(i + 1) * 128],
                                 wvs, start=(st_ and i == 0), stop=sp_)

        def epilogue_stage(ps, c):
            """All ps readers up front so the PSUM set releases quickly."""
            msqs = []
            for i in range(2):
                msq = spool.tile([128, 1], F32, name=f"msq{i}", tag=f"msq{i}")
                vsq = tpool.tile([128, D], F32, name="vsq", tag="hA0")
                nc.scalar.activation(out=vsq, in_=ps[:, 6 + i, :],
                                     func=AF.Square, accum_out=msq)
                msqs.append(msq)
            psts = []
            for gi in range(3):
                pst = tpool.tile([128, 2, 256], F32, name=f"pst{gi}",
                                 tag=f"pst{gi}")
                nc.scalar.copy(out=pst, in_=ps[:, 2 * gi:2 * gi + 2, :])
                psts.append(pst)
            for i in range(2):
                sdv = spool.tile([128, 1], F32, tag="sdv")
                nc.scalar.activation(out=sdv, in_=msqs[i], func=AF.Sqrt,
                                     scale=1.0 / D, bias=epsb)
                rvv = spool.tile([128, 1], F32, tag="rvv")
                nc.vector.reciprocal(out=rvv, in_=sdv)
                nc.vector.tensor_scalar_mul(V[:, 2 * c + i, :],
                                            ps[:, 6 + i, :], rvv)
            return psts

        def epilogue_heads(c, csn, psts):
            """Norm/rope chain from the SBUF staging tiles.

            Breadth-first emission: same-stage ops across the three head
            groups go out together so the in-order engine queues execute
            the groups concurrently instead of serializing the chains.
            """
            sl = slice(c * 256, (c + 1) * 256)
            heads = [(QT[0], QT[1], 0, qw1p), (QT[2], QT[3], 1, qw1p),
                     (KTt[0], KTt[1], 2, kw1p)]
            slA, slB, slC, slD = [
                [tpool.tile([128, 256], F32, name=f"{nm}{g}", tag=f"{nm}{g}")
                 for g in range(3)] for nm in ("hA", "hB", "hC", "hD")]
            for g in range(3):
                nc.scalar.activation(out=slA[g], in_=psts[g][:, 0, :], func=AF.Square)
                nc.scalar.activation(out=slB[g], in_=psts[g][:, 1, :], func=AF.Square)
            for g in range(3):
                nc.gpsimd.partition_all_reduce(slC[g], slA[g], channels=128,
                                               reduce_op=bass_isa.ReduceOp.add)
                nc.gpsimd.partition_all_reduce(slD[g], slB[g], channels=128,
                                               reduce_op=bass_isa.ReduceOp.add)
            ssq, sd, rv, qnA, qnB = [], [], [], [], []
            for g in range(3):      # ssq -> slot A
                t = tpool.tile([128, 256], F32, name=f"ssq{g}", tag=f"hA{g}")
                nc.vector.tensor_add(t, slC[g], slD[g])
                ssq.append(t)
            for g in range(3):      # sd -> slot B
                t = tpool.tile([128, 256], F32, name=f"sd{g}", tag=f"hB{g}")
                nc.scalar.activation(out=t, in_=ssq[g], func=AF.Sqrt,
                                     scale=1.0 / D, bias=epsb)
                sd.append(t)
            for g in range(3):      # rv -> slot C
                t = tpool.tile([128, 256], F32, name=f"rv{g}", tag=f"hC{g}")
                nc.vector.reciprocal(out=t, in_=sd[g])
                rv.append(t)
            for g, (dstA, dstB, gi, w1p) in enumerate(heads):
                a = tpool.tile([128, 256], F32, name=f"qnA{g}", tag=f"hD{g}")
                nc.vector.scalar_tensor_tensor(
                    out=a, in0=psts[g][:, 0, :], scalar=w1p[:, 0:1],
                    in1=rv[g], op0=AL.mult, op1=AL.mult)
                qnA.append(a)
                b_ = tpool.tile([128, 256], F32, name=f"qnB{g}", tag=f"hA{g}")
                nc.vector.scalar_tensor_tensor(
                    out=b_, in0=psts[g][:, 1, :], scalar=w1p[:, 1:2],
                    in1=rv[g], op0=AL.mult, op1=AL.mult)
                qnB.append(b_)
            t1s, t2s = [], []
            for g in range(3):
                t1 = tpool.tile([128, 256], F32, name=f"t1{g}", tag=f"hB{g}")
                nc.vector.tensor_mul(t1, qnA[g], csn[:, 0, :])
                t1s.append(t1)
                t2 = tpool.tile([128, 256], F32, name=f"t2{g}", tag=f"hC{g}")
                nc.gpsimd.tensor_mul(t2, qnB[g], csn[:, 2, :])
                t2s.append(t2)
            for g, (dstA, dstB, gi, w1p) in enumerate(heads):
                nc.vector.tensor_sub(dstA[:, sl], t1s[g], t2s[g])
            t3s, t4s = [], []
            for g in range(3):
                t3 = tpool.tile([128, 256], F32, name=f"t3{g}", tag=f"hB{g}")
                nc.vector.tensor_mul(t3, qnB[g], csn[:, 1, :])
                t3s.append(t3)
                t4 = tpool.tile([128, 256], F32, name=f"t4{g}", tag=f"hC{g}")
                nc.gpsimd.tensor_mul(t4, qnA[g], csn[:, 3, :])
                t4s.append(t4)
            for g, (dstA, dstB, gi, w1p) in enumerate(heads):
                nc.vector.tensor_add(dstB[:, sl], t3s[g], t4s[g])

        def epilogue(ps, c, csn):
            epilogue_heads(c, csn, epilogue_stage(ps, c))

        def load_csn(c):
            # issued from the ACT queue: a pool-reuse wait here must not
            # stall the SP DMA stream feeding the matmuls
            sl = slice(c * 256, (c + 1) * 256)
            csn = cpool.tile([128, 4, 256], F32, name="csn", tag="csn")
            nc.scalar.dma_start(out=csn[:, 0:2, :], in_=cos_d[:, :, sl])
            nc.scalar.dma_start(out=csn[:, 2:4, :], in_=sin_d[:, :, sl])
            return csn

        # merged chunks 0+1 (paced by the startup DMA stream)
        csn0 = load_csn(0)
        csn1 = load_csn(1)
        ps0 = pps.tile([128, 8, 256], F32, name="ps0", tag="ps")
        ps1 = pps.tile([128, 8, 256], F32, name="ps1", tag="ps")
        # chunk 0/1 quarters all issued in the startup stream above; only
        # chunk 2's first quarters start here
        merged_loads = {15: [(2, 0)], 17: [(2, 1)]}
        for kt in range(KT):
            for (cc, qq) in merged_loads.get(kt, ()):
                load_hq(cc, qq)
            proj_matmuls(ps0, 0, kt)
            proj_matmuls(ps1, 1, kt)
            if kt % 5 == 4:
                free_hq(0, kt // 5)
                free_hq(1, kt // 5)
        psts0 = epilogue_stage(ps0, 0)
        epilogue_heads(0, csn0, psts0)
        psts1 = epilogue_stage(ps1, 1)
        heads_pending = (1, csn1, psts1)

        def chunk_loop(ps, c):
            for kt in range(KT):
                if kt == 0:
                    load_hq(c, 3)
                    if c == 2:
                        load_hq(c, 2)
                elif c + 1 < NC256:
                    if kt == 5:
                        load_hq(c + 1, 0)
                    elif kt == 10:
                        load_hq(c + 1, 1)
                    elif kt == 15:
                        load_hq(c + 1, 2)
                proj_matmuls(ps, c, kt)
                if kt % 5 == 4:
                    free_hq(c, kt // 5)

        for c in range(2, 8):
            csn = load_csn(c)
            ps = pps.tile([128, 8, 256], F32, name="ps", tag="ps")
            chunk_loop(ps, c)
            # previous chunk's norm/rope chain drains during this chunk's
            # matmuls; this chunk's PSUM staging lands right behind it
            epilogue_heads(*heads_pending)
            psts = epilogue_stage(ps, c)
            heads_pending = (c, csn, psts)
        pP.close()

        # phase-2 PSUM pools; the heads part of chunk 7's epilogue drains on
        # DVE/ACT/Pool while the first attention rounds start
        p2b = ExitStack()
        scps = p2b.enter_context(tc.tile_pool(name="scps", bufs=1, space="PSUM"))
        avps = p2b.enter_context(tc.tile_pool(name="avps", bufs=1, space="PSUM"))
        trps = p2b.enter_context(tc.tile_pool(name="trps", bufs=2, space="PSUM"))
        wops = p2b.enter_context(tc.tile_pool(name="wops", bufs=2, space="PSUM"))

        it_ctr = [0]

        def emit_scores_exp(t, h, early=False):
            """Scores + row-max + exp for (t, h); generator with stage marks.

            Early (single-chunk) rounds take their score bank from the
            still-idle wops pool so two rounds can pipeline before the main
            pools open."""
            r = it_ctr[0] % 2
            it_ctr[0] += 1
            w0, ch, pad = chunk_plan(t)
            assert not early or len(ch) == 1
            wt = min(t, 8) + 1
            if early:
                sc2 = wops.tile([128, 512], F32, name="scsE", tag="wop")
                chunk_ap = lambda ci: sc2
            else:
                scs = scps.tile([128, 3, 512], F32, name="scs", tag="scs")
                chunk_ap = lambda ci: scs[:, ci]
            nms = []
            off = 0
            for ci, nt in enumerate(ch):
                width = nt * 128
                ca = chunk_ap(ci)
                mlist = []
                if t >= 8 and ci == 0:
                    mlist.append((0, MK_BOUND))
                dlocal = (wt - 1) - off
                if 0 <= dlocal < nt:
                    mlist.append((dlocal, MK_DIAG))
                for p_ in range(nt):
                    if off + p_ >= wt:
                        mlist.append((p_, MK_FULL))
                first = True
                for (ti, kind) in mlist:
                    nc.tensor.matmul(
                        ca[:, ti * 128:(ti + 1) * 128],
                        identb, m3[:, kind, :], start=first, stop=False)
                    first = False
                kcol = (w0 + off) * 128
                for j in range(NDK):
                    nc.tensor.matmul(
                        ca[:, 0:width],
                        QT[2 * h + j][:, t * 128:(t + 1) * 128],
                        KTt[j][:, kcol:kcol + width],
                        start=(first and j == 0), stop=(j == NDK - 1))
                off += nt
                # row-max: batch the leading equal-width chunk pair
                if not early and ci == 1 and ch[0] == ch[1]:
                    nm = big.tile([128, 1], F32, name=f"nm0_{r}", tag=f"nm0_{r}")
                    nc.vector.tensor_reduce(out=nm, in_=scs[:, 0:2, 0:width],
                                            axis=AX.XY, op=AL.max, negate=True)
                    nms = [nm]
                elif not (not early and ci == 0 and len(ch) > 1 and ch[0] == ch[1]):
                    nm = big.tile([128, 1], F32, name=f"nm{ci}_{r}", tag=f"nm{ci}_{r}")
                    nc.vector.tensor_reduce(out=nm, in_=ca[:, 0:width],
                                            axis=AX.X, op=AL.max, negate=True)
                    nms.append(nm)
            yield "scores_done"
            negm = nms[0]
            if len(nms) > 1:
                negm = big.tile([128, 1], F32, name=f"negm_{r}", tag=f"negm_{r}")
                nc.vector.tensor_tensor(negm, nms[0], nms[1], op=AL.min)
                if len(nms) > 2:
                    nc.vector.tensor_tensor(negm, negm, nms[2], op=AL.min)
            yield "negm_done"
            if early:
                expf = big.tile([128, 384], F32R, name=f"expbE_{r}",
                                tag=f"expbE_{r}")
            else:
                expb = etpool.tile([128, 3, 384], F32R, name="expb", tag="expb")
                expf = expb.rearrange("p a b -> p (a b)")
            den = big.tile([128, 3], F32, name=f"den_{r}", tag=f"den_{r}")
            off = 0
            for ci, nt in enumerate(ch):
                width = nt * 128
                nc.scalar.activation(
                    out=expf[:, off * 128:off * 128 + width],
                    in_=chunk_ap(ci)[:, 0:width], func=AF.Exp, bias=negm,
                    accum_out=den[:, ci:ci + 1])
                off += nt
            if debug and t == 10 and h == 0:
                nc.sync.dma_start(out=dbg["dexpb"][:, 0:1152],
                                  in_=expf.bitcast(F32))
                nc.sync.dma_start(out=dbg["dnegm"][:, :], in_=negm)
                nc.sync.dma_start(out=dbg["dden"][:, :], in_=den)
            if len(ch) == 1:
                dsv = den[:, 0:1]
            else:
                dsum = big.tile([128, 1], F32, name=f"dsum_{r}", tag=f"dsum_{r}")
                nc.vector.tensor_reduce(out=dsum, in_=den[:, 0:len(ch)],
                                        axis=AX.X, op=AL.add)
                dsv = dsum
            rden = big.tile([128, 1], F32, name=f"rden_{r}", tag=f"rden_{r}")
            nc.vector.reciprocal(out=rden, in_=dsv)
            yield (expf, rden, ch, w0)

        def run_gen(g, until):
            while True:
                v = next(g)
                if v == until or not isinstance(v, str):
                    return v

        cp_rr = [0]

        def psum_copy(dst, src):
            e = (0, 1, 0)[cp_rr[0] % 3]      # 2:1 DVE:ACT
            cp_rr[0] += 1
            if e == 0:
                nc.vector.tensor_copy(out=dst, in_=src)
            else:
                nc.scalar.copy(out=dst, in_=src)

        def emit_attn_core(state, trpool, avpool, expT, ao_dst):
            """Transposes, AV matmul, and attn-out scale into ao_dst."""
            expf, rden, ch, w0 = state
            T = sum(ch)
            expTf = expT.rearrange("p a b -> p (a b)")
            off = 0
            for ci, nt in enumerate(ch):
                width = nt * 128
                trp = trpool.tile([128, 512], F32R, name="trp", tag="trp")
                for q_ in range(nt):
                    nc.tensor.transpose(
                        trp[:, q_ * 128:(q_ + 1) * 128],
                        expf[:, (off + q_) * 128:(off + q_ + 1) * 128], ident)
                psum_copy(expTf[:, off * 128:off * 128 + width], trp[:, 0:width])
                off += nt
            avp = avpool.tile([128, D], F32, name="avp", tag="avp")
            for a in range(T):
                nc.tensor.matmul(avp, expT[:, a, :], V[:, w0 + a, :],
                                 start=(a == 0), stop=(a == T - 1))
            nc.scalar.activation(out=ao_dst, in_=avp, func=AF.Copy, scale=rden)

        # boundary rounds t=0..NEARLY-1: scores/exp/AV run while the chunk-7
        # heads chain drains; attn-out goes to big-pool slots, aoT deferred
        first = emit_scores_exp(0, 0, early=True)
        run_gen(first, "scores_done")
        epilogue_heads(*heads_pending)
        st00 = run_gen(first, None)
        emit_attn_core(st00, trps, avps, expTE, aoE[0])
        for t in range(NEARLY):
            for h in range(2):
                if t == 0 and h == 0:
                    continue
                g = emit_scores_exp(t, h, early=True)
                run_gen(g, "scores_done")
                st = run_gen(g, None)
                emit_attn_core(st, trps, avps, expTE, aoE[2 * t + h])
        pA.close()

        # ---------------- Phase 2 main + output projection ----------------
        etpool = p2b.enter_context(tc.tile_pool(name="etpool", bufs=2))
        wopool = p2b.enter_context(tc.tile_pool(name="wopool", bufs=1))
        aopool = p2b.enter_context(tc.tile_pool(name="aopool", bufs=1))
        opool = p2b.enter_context(tc.tile_pool(name="opool", bufs=2))

        woT = wopool.tile([128, NDQ, H], F32R, tag="woT")
        for dj in range(NDQ):
            nc.sync.dma_start(out=woT[:, dj:dj + 1, :], in_=wo_d[:, dj:dj + 1, :])
        aoT = [aopool.tile([128, S], F32R, name=f"aoT{j}", tag=f"aoT{j}")
               for j in range(NDQ)]

        wo_queue = []
        ost_tiles = {}

        def emit_wo_part(min_backlog=0):
            """Emit one Wo output column chunk (4 matmuls + copy) from the
            backlog; fine granules let PE fill sub-us dependency holes."""
            if len(wo_queue) <= min_backlog:
                return
            t, hc = wo_queue.pop(0)
            if hc == 0:
                ost_tiles[t] = opool.tile([128, H], F32, name="ost", tag="ost")
            ost = ost_tiles[t]
            wop = wops.tile([128, 512], F32, name="wop", tag="wop")
            for dj in range(NDQ):
                nc.tensor.matmul(
                    wop, aoT[dj][:, t * 128:(t + 1) * 128],
                    woT[:, dj, hc * 512:(hc + 1) * 512],
                    start=(dj == 0), stop=(dj == NDQ - 1))
            psum_copy(ost[:, hc * 512:(hc + 1) * 512], wop)
            if hc == 2:
                nc.sync.dma_start(
                    out=out_d[t * 128:(t + 1) * 128, 0:1536],
                    in_=ost[:, 0:1536])
            elif hc == 4:
                nc.sync.dma_start(
                    out=out_d[t * 128:(t + 1) * 128, 1536:H],
                    in_=ost_tiles.pop(t)[:, 1536:H])

        def emit_aoT(t, h, ao_src):
            trp = trps.tile([128, 512], F32R, name="trpa", tag="trp")
            for j in range(2):
                nc.tensor.transpose(trp[:, j * 128:(j + 1) * 128],
                                    ao_src[:, j * 128:(j + 1) * 128], ident)
            for j in range(2):
                psum_copy(aoT[2 * h + j][:, t * 128:(t + 1) * 128],
                          trp[:, j * 128:(j + 1) * 128])

        # catch up: aoT for the early rounds, then queue their Wo parts
        for t in range(NEARLY):
            for h in range(2):
                emit_aoT(t, h, aoE[2 * t + h])
            wo_queue += [(t, hc) for hc in range(5)]
            if t >= 2:
                emit_wo_part()
                emit_wo_part()

        for t in range(NEARLY, ST):
            for h in range(2):
                g = emit_scores_exp(t, h)
                run_gen(g, "scores_done")
                emit_wo_part()
                emit_wo_part(min_backlog=4)
                run_gen(g, "negm_done")
                emit_wo_part(min_backlog=2)
                state = run_gen(g, None)
                ao = etpool.tile([128, D], F32R, name="ao", tag="ao")
                expT = etpool.tile([128, 9, 128], F32R, name="expT", tag="expT")
                emit_attn_core(state, trps, avps, expT, ao)
                emit_wo_part(min_backlog=6)
                emit_aoT(t, h, ao)
            wo_queue += [(t, hc) for hc in range(5)]
            emit_wo_part(min_backlog=8)

        while wo_queue:
            emit_wo_part()

        p2b.close()

    nc.compile()
    return nc


_nc_cache = None


def _prep_inputs(hidden_states, attention_mask, cos, sin, Wq, Wk, Wv, Wo,
                 q_norm_w, k_norm_w):
    import ml_dtypes
    bf16 = ml_dtypes.bfloat16

    hidden_states = np.asarray(hidden_states, dtype=np.float32)
    mask = np.asarray(attention_mask, dtype=np.float32)[0, 0]      # [S, S]
    cos2 = np.asarray(cos, dtype=np.float32)[0, 0]                 # [S, D]
    sin2 = np.asarray(sin, dtype=np.float32)[0, 0]
    Wq = np.asarray(Wq, dtype=np.float32)
    Wk = np.asarray(Wk, dtype=np.float32)
    Wv = np.asarray(Wv, dtype=np.float32)
    Wo = np.asarray(Wo, dtype=np.float32)

    cosh = np.ascontiguousarray(
        cos2.T.reshape(NDK, 128, S).transpose(1, 0, 2))            # [128,2,S]
    sinh = np.ascontiguousarray(sin2.T.reshape(NDK, 128, S).transpose(1, 0, 2))

    diag = mask[0:128, 0:128]
    bound = mask[1024:1152, 0:128]
    full = mask[0:128, 1024:1152]
    m3h = np.ascontiguousarray(
        np.stack([diag, bound, full], axis=1)).astype(bf16)        # [128,3,128]

    qw1p = np.ascontiguousarray(
        (1.0 + np.asarray(q_norm_w, dtype=np.float32)).reshape(NDK, 128).T)
    kw1p = np.ascontiguousarray(
        (1.0 + np.asarray(k_norm_w, dtype=np.float32)).reshape(NDK, 128).T)
    ident = round_f32r(np.eye(128, dtype=np.float32))
    identb = np.eye(128, dtype=np.float32).astype(bf16)

    in_maps = []
    for core in range(8):
        b, g = core // 4, core % 4
        hsT = round_f32r(np.ascontiguousarray(hidden_states[b].T))  # [H, S]
        hsC = np.ascontiguousarray(
            hsT.reshape(KT, 128, NC256, 256).transpose(2, 1, 0, 3))
        wqh = round_f32r(np.ascontiguousarray(
            Wq[2 * g * D:(2 * g + 2) * D].T.reshape(KT, 128, DQ).transpose(1, 0, 2)))
        wkh = round_f32r(np.ascontiguousarray(
            Wk[g * D:(g + 1) * D].T.reshape(KT, 128, D).transpose(1, 0, 2)))
        wvh = round_f32r(np.ascontiguousarray(
            Wv[g * D:(g + 1) * D].T.reshape(KT, 128, D).transpose(1, 0, 2)))
        woh = round_f32r(np.ascontiguousarray(
            Wo[:, 2 * g * D:(2 * g + 2) * D].T.reshape(NDQ, 128, H).transpose(1, 0, 2)))
        w0pack = np.ascontiguousarray(
            np.concatenate([wqh[:, 0, :], wkh[:, 0, :], wvh[:, 0, :]],
                           axis=1).reshape(128, 4, 256))
        in_maps.append({
            "hsC": hsC, "wqh": wqh, "wkh": wkh, "wvh": wvh, "woh": woh,
            "cosh": cosh, "sinh": sinh, "m3h": m3h, "ident": ident,
            "identb": identb, "qw1p": qw1p, "kw1p": kw1p, "w0pack": w0pack,
        })
    return in_maps


def kernel(hidden_states, attention_mask, cos, sin, Wq, Wk, Wv, Wo,
           q_norm_w, k_norm_w):
    global _nc_cache
    if _nc_cache is None:
        _nc_cache = build_nc()
    nc = _nc_cache

    in_maps = _prep_inputs(hidden_states, attention_mask, cos, sin,
                           Wq, Wk, Wv, Wo, q_norm_w, k_norm_w)
    res = run_bass_kernel_spmd(nc, in_maps, core_ids=list(range(8)))
    outs = [r["out"] for r in res.results]
    final = np.zeros((B, S, H), dtype=np.float32)
    for core in range(8):
        final[core // 4] += outs[core]
    return final


# revision 5
# speedup vs baseline: 1.0702x; 1.0096x over previous
"""Gemma-style sliding-window attention block on 8 trn2 NeuronCores, v2.

Sharding: tensor-parallel over kv-head groups (4) x data-parallel over
batch (2).  Core c handles batch b = c//4 and kv-head g = c%4 (query
heads 2g, 2g+1).  Host sums the 4 partial Wo outputs per batch.

Structure:
- Phase 1 runs in 8 seq-chunks of 256 with a double-buffered 4-bank PSUM
  set; the first two chunks share a merged kt loop paced by the initial
  weight-stream DMAs.  cos/sin chunk loads are issued from the ACT queue
  so their pool-reuse waits cannot stall the SP DMA stream.
- RMS-norm partition sums and all rope multiplies run on gpsimd (SBUF
  only; the BIR verifier forbids gpsimd PSUM access).
- Attention skips fully-masked future key tiles for t<8 and initializes
  score PSUM banks with constant mask tiles (mask-first matmul).
- Attention rounds for t=0..5 run from a small early PSUM pool that
  coexists with the last projection chunk, overlapping its epilogue;
  their attn-out transposes are caught up afterwards.
- Wo projection matmuls are interleaved into the attention loop as PE
  filler work, drained from a backlog queue.
"""
import numpy as np
from contextlib import ExitStack

import concourse.bass as bass
import concourse.bacc as bacc
import concourse.mybir as mybir
import concourse.tile as tile
from concourse import bass_isa
from concourse.bass_utils import run_bass_kernel_spmd

F32 = mybir.dt.float32
F32R = mybir.dt.float32r
BF16 = mybir.dt.bfloat16
AL = mybir.AluOpType
AF = mybir.ActivationFunctionType
AX = mybir.AxisListType

B, S, H = 2, 2048, 2560
NH, NKV, D = 8, 4, 256
SW = 1024
EPS = 1e-6
ST = S // 128             # 16 q/k tiles
KT = H // 128             # 20 hidden k-tiles
NC256 = S // 256          # 8 seq chunks (phase 1)
DQ = 2 * D                # per-core query dims (2 heads)
NDQ = DQ // 128           # 4
NDK = D // 128            # 2
NEARLY = 3                # q-tiles whose attn-out/Wo work is deferred past
                          # the phase-1 SBUF pool barrier

MK_DIAG, MK_BOUND, MK_FULL = 0, 1, 2


def round_f32r(x: np.ndarray) -> np.ndarray:
    """Round fp32 to f32r (11-bit mantissa, round-to-nearest-even)."""
    b = np.ascontiguousarray(x, dtype=np.float32).view(np.uint32).astype(np.uint64)
    bias = 0x7FF + ((b >> 12) & 1)
    return ((b + bias) & 0xFFFFF000).astype(np.uint32).view(np.float32)


def chunk_plan(t):
    """(w0, [ntile per chunk], pad) for q-tile t. Chunks are 2-3 tiles so
    every matmul free dim >= 256 (full f32r rate); t=0 pads one dead tile."""
    w0 = max(0, t - 8)
    wt = min(t, 8) + 1
    plans = {1: [2], 2: [2], 3: [3], 4: [2, 2], 5: [3, 2], 6: [3, 3],
             7: [3, 2, 2], 8: [3, 3, 2], 9: [3, 3, 3]}
    ch = plans[wt]
    return w0, ch, sum(ch) - wt


def build_nc(debug=False):
    nc = bacc.Bacc("TRN2", target_bir_lowering=False, debug=False)

    hs_d = nc.dram_tensor("hsC", [NC256, 128, KT, 256], F32R, kind="ExternalInput")
    wq_d = nc.dram_tensor("wqh", [128, KT, DQ], F32R, kind="ExternalInput")
    wk_d = nc.dram_tensor("wkh", [128, KT, D], F32R, kind="ExternalInput")
    wv_d = nc.dram_tensor("wvh", [128, KT, D], F32R, kind="ExternalInput")
    wo_d = nc.dram_tensor("woh", [128, NDQ, H], F32R, kind="ExternalInput")
    cos_d = nc.dram_tensor("cosh", [128, NDK, S], F32, kind="ExternalInput")
    sin_d = nc.dram_tensor("sinh", [128, NDK, S], F32, kind="ExternalInput")
    m3_d = nc.dram_tensor("m3h", [128, 3, 128], BF16, kind="ExternalInput")
    idn_d = nc.dram_tensor("ident", [128, 128], F32R, kind="ExternalInput")
    idb_d = nc.dram_tensor("identb", [128, 128], BF16, kind="ExternalInput")
    qw_d = nc.dram_tensor("qw1p", [128, NDK], F32, kind="ExternalInput")
    w0p_d = nc.dram_tensor("w0pack", [128, 4, 256], F32R, kind="ExternalInput")
    kw_d = nc.dram_tensor("kw1p", [128, NDK], F32, kind="ExternalInput")
    out_d = nc.dram_tensor("out", [S, H], F32, kind="ExternalOutput")
    dbg = {}
    if debug:
        for nm, shp in [("dQT0", [128, S]), ("dKT0", [128, S]),
                        ("dV", [128, ST * D]), ("dexpb", [128, 1152]),
                        ("dnegm", [128, 1]), ("dden", [128, 3])]:
            dbg[nm] = nc.dram_tensor(nm, shp, F32, kind="ExternalOutput")

    with ExitStack() as top:
        tc = top.enter_context(tile.TileContext(nc))
        big = top.enter_context(tc.tile_pool(name="big", bufs=1))

        # Whole-kernel residents
        QT = [big.tile([128, S], F32R, name=f"QT{j}", tag=f"QT{j}") for j in range(NDQ)]
        KTt = [big.tile([128, S], F32R, name=f"KTt{j}", tag=f"KTt{j}") for j in range(NDK)]
        V = big.tile([128, ST, D], F32R, tag="V")
        ident = big.tile([128, 128], F32R, tag="ident")
        identb = big.tile([128, 128], BF16, tag="identb")
        m3 = big.tile([128, 3, 128], BF16, tag="m3")
        epsb = big.tile([128, 1], F32, tag="epsb")
        qw1p = big.tile([128, NDK], F32, tag="qw1p")
        kw1p = big.tile([128, NDK], F32, tag="kw1p")
        # boundary-overlap staging: attn-out and P/P^T tiles for t<NEARLY
        aoE = [big.tile([128, D], F32R, name=f"aoE{i}", tag=f"aoE{i}")
               for i in range(2 * NEARLY)]
        expTE = big.tile([128, 3, 128], F32R, name="expTE", tag="expTE")

        # ---------------- Phase 1: projections + norms + rope -------------
        pA = ExitStack()        # phase-1 SBUF pools
        wpool = pA.enter_context(tc.tile_pool(name="wpool", bufs=1))
        hpool = pA.enter_context(tc.tile_pool(name="hpool", bufs=5))
        cpool = pA.enter_context(tc.tile_pool(name="cpool", bufs=2))
        tpool = pA.enter_context(tc.tile_pool(name="tpool", bufs=1))
        spool = pA.enter_context(tc.tile_pool(name="spool", bufs=2))
        pP = ExitStack()        # phase-1 PSUM (chunks 0..6)
        pps = pP.enter_context(tc.tile_pool(name="pps", bufs=2, space="PSUM"))

        wq = wpool.tile([128, KT - 1, DQ], F32R, tag="wq")
        wk = wpool.tile([128, KT - 1, D], F32R, tag="wk")
        wv = wpool.tile([128, KT - 1, D], F32R, tag="wv")

        # hs chunk tiles in 5-kt quarters
        hquarts = {}

        def load_hq(c, q):
            ht = hpool.tile([128, 5, 256], F32R, name="hst", tag="hst")
            nc.sync.dma_start(out=ht, in_=hs_d[c, :, q * 5:(q + 1) * 5, :])
            hquarts[(c, q)] = ht

        def get_hst(c, kt):
            return hquarts[(c, kt // 5)][:, kt % 5, :]

        def free_hq(c, q):
            hquarts.pop((c, q), None)

        # staged startup: one packed first-kt weight DMA so matmuls start
        # as early as possible; kt==0 matmuls for every chunk read w0pack.
        w0pack = wpool.tile([128, 4, 256], F32R, tag="w0pack")
        w0f = w0pack.rearrange("p a b -> p (a b)")
        nc.sync.dma_start(out=w0pack, in_=w0p_d[:, :, :])
        ht00 = hpool.tile([128, 5, 256], F32R, name="hst", tag="hst")
        nc.sync.dma_start(out=ht00[:, 0:1, :], in_=hs_d[0, :, 0:1, :])
        ht10 = hpool.tile([128, 5, 256], F32R, name="hst", tag="hst")
        nc.sync.dma_start(out=ht10[:, 0:1, :], in_=hs_d[1, :, 0:1, :])
        nc.sync.dma_start(out=ht00[:, 1:5, :], in_=hs_d[0, :, 1:5, :])
        nc.sync.dma_start(out=ht10[:, 1:5, :], in_=hs_d[1, :, 1:5, :])
        hquarts[(0, 0)] = ht00
        hquarts[(1, 0)] = ht10
        nc.sync.dma_start(out=wq[:, 0:3, :], in_=wq_d[:, 1:4, :])
        nc.sync.dma_start(out=wk[:, 0:5, :], in_=wk_d[:, 1:6, :])
        nc.sync.dma_start(out=wv[:, 0:5, :], in_=wv_d[:, 1:6, :])
        nc.sync.dma_start(out=wq[:, 3:6, :], in_=wq_d[:, 4:7, :])
        load_hq(0, 1)
        load_hq(1, 1)
        nc.sync.dma_start(out=wq[:, 6:9, :], in_=wq_d[:, 7:10, :])
        nc.sync.dma_start(out=wk[:, 5:12, :], in_=wk_d[:, 6:13, :])
        nc.sync.dma_start(out=wv[:, 5:12, :], in_=wv_d[:, 6:13, :])
        load_hq(0, 2)
        nc.sync.dma_start(out=wq[:, 9:13, :], in_=wq_d[:, 10:14, :])
        load_hq(1, 2)
        nc.sync.dma_start(out=wk[:, 12:19, :], in_=wk_d[:, 13:20, :])
        nc.sync.dma_start(out=wv[:, 12:19, :], in_=wv_d[:, 13:20, :])
        load_hq(0, 3)
        nc.sync.dma_start(out=wq[:, 13:16, :], in_=wq_d[:, 14:17, :])
        load_hq(1, 3)
        nc.sync.dma_start(out=wq[:, 16:19, :], in_=wq_d[:, 17:20, :])
        nc.sync.dma_start(out=ident, in_=idn_d[:, :])
        nc.sync.dma_start(out=identb, in_=idb_d[:, :])
        nc.sync.dma_start(out=m3, in_=m3_d[:, :, :])
        nc.sync.dma_start(out=qw1p, in_=qw_d[:, :])
        nc.sync.dma_start(out=kw1p, in_=kw_d[:, :])
        nc.vector.memset(epsb, EPS)
        actwarm = big.tile([128, 1], F32, tag="actwarm")
        for fn in (AF.Square, AF.Sqrt, AF.Exp, AF.Copy):
            nc.scalar.activation(out=actwarm, in_=epsb, func=fn)

        def proj_matmuls(ps, c, kt):
            hst = get_hst(c, kt)
            st_, sp_ = (kt == 0), (kt == KT - 1)
            if kt == 0:
                wqs = lambda j: w0f[:, j * 128:(j + 1) * 128]
                wks = lambda j: w0f[:, 512 + j * 128:512 + (j + 1) * 128]
                wvs = w0f[:, 768:1024]
            else:
                wqs = lambda j: wq[:, kt - 1, j * 128:(j + 1) * 128]
                wks = lambda j: wk[:, kt - 1, j * 128:(j + 1) * 128]
                wvs = wv[:, kt - 1, :]
            for j in range(NDQ):
                nc.tensor.matmul(ps[:, j, :], wqs(j),
                                 hst, start=(st_ and j % 2 == 0), stop=sp_)
            for j in range(NDK):
                nc.tensor.matmul(ps[:, 4 + j, :], wks(j),
                                 hst, start=(st_ and j == 0), stop=sp_)
            for i in range(2):
                nc.tensor.matmul(ps[:, 6 + i, :], hst[:, i * 128:(i + 1) * 128],
                                 wvs, start=(st_ and i == 0), stop=sp_)

        def epilogue_stage(ps, c):
            """All ps readers up front so the PSUM set releases quickly."""
            msqs = []
            for i in range(2):
                msq = spool.tile([128, 1], F32, name=f"msq{i}", tag=f"msq{i}")
                vsq = tpool.tile([128, D], F32, name="vsq", tag="hA0")
                nc.scalar.activation(out=vsq, in_=ps[:, 6 + i, :],
                                     func=AF.Square, accum_out=msq)
                msqs.append(msq)
            psts = []
            for gi in range(3):
                pst = tpool.tile([128, 2, 256], F32, name=f"pst{gi}",
                                 tag=f"pst{gi}")
                nc.scalar.copy(out=pst, in_=ps[:, 2 * gi:2 * gi + 2, :])
                psts.append(pst)
            for i in range(2):
                sdv = spool.tile([128, 1], F32, tag="sdv")
                nc.scalar.activation(out=sdv, in_=msqs[i], func=AF.Sqrt,
                                     scale=1.0 / D, bias=epsb)
                rvv = spool.tile([128, 1], F32, tag="rvv")
                nc.vector.reciprocal(out=rvv, in_=sdv)
                nc.vector.tensor_scalar_mul(V[:, 2 * c + i, :],
                                            ps[:, 6 + i, :], rvv)
            return psts

        def epilogue_heads(c, csn, psts):
            """Norm/rope chain from the SBUF staging tiles.

            Breadth-first emission: same-stage ops across the three head
            groups go out together so the in-order engine queues execute
            the groups concurrently instead of serializing the chains.
            """
            sl = slice(c * 256, (c + 1) * 256)
            heads = [(QT[0], QT[1], 0, qw1p), (QT[2], QT[3], 1, qw1p),
                     (KTt[0], KTt[1], 2, kw1p)]
            slA, slB, slC, slD = [
                [tpool.tile([128, 256], F32, name=f"{nm}{g}", tag=f"{nm}{g}")
                 for g in range(3)] for nm in ("hA", "hB", "hC", "hD")]
            for g in range(3):
                nc.scalar.activation(out=slA[g], in_=psts[g][:, 0, :], func=AF.Square)
                nc.scalar.activation(out=slB[g], in_=psts[g][:, 1, :], func=AF.Square)
            for g in range(3):
                nc.gpsimd.partition_all_reduce(slC[g], slA[g], channels=128,
                                               reduce_op=bass_isa.ReduceOp.add)
                nc.gpsimd.partition_all_reduce(slD[g], slB[g], channels=128,
                                               reduce_op=bass_isa.ReduceOp.add)
            ssq, sd, rv, qnA, qnB = [], [], [], [], []
            for g in range(3):      # ssq -> slot A
                t = tpool.tile([128, 256], F32, name=f"ssq{g}", tag=f"hA{g}")
                nc.vector.tensor_add(t, slC[g], slD[g])
                ssq.append(t)
            for g in range(3):      # sd -> slot B
                t = tpool.tile([128, 256], F32, name=f"sd{g}", tag=f"hB{g}")
                nc.scalar.activation(out=t, in_=ssq[g], func=AF.Sqrt,
                                     scale=1.0 / D, bias=epsb)
                sd.append(t)
            for g in range(3):      # rv -> slot C
                t = tpool.tile([128, 256], F32, name=f"rv{g}", tag=f"hC{g}")
                nc.vector.reciprocal(out=t, in_=sd[g])
                rv.append(t)
            for g, (dstA, dstB, gi, w1p) in enumerate(heads):
                a = tpool.tile([128, 256], F32, name=f"qnA{g}", tag=f"hD{g}")
                nc.vector.scalar_tensor_tensor(
                    out=a, in0=psts[g][:, 0, :], scalar=w1p[:, 0:1],
                    in1=rv[g], op0=AL.mult, op1=AL.mult)
                qnA.append(a)
                b_ = tpool.tile([128, 256], F32, name=f"qnB{g}", tag=f"hA{g}")
                nc.vector.scalar_tensor_tensor(
                    out=b_, in0=psts[g][:, 1, :], scalar=w1p[:, 1:2],
                    in1=rv[g], op0=AL.mult, op1=AL.mult)
                qnB.append(b_)
            t1s, t2s = [], []
            for g in range(3):
                t1 = tpool.tile([128, 256], F32, name=f"t1{g}", tag=f"hB{g}")
                nc.vector.tensor_mul(t1, qnA[g], csn[:, 0, :])
                t1s.append(t1)
                t2 = tpool.tile([128, 256], F32, name=f"t2{g}", tag=f"hC{g}")
                nc.gpsimd.tensor_mul(t2, qnB[g], csn[:, 2, :])
                t2s.append(t2)
            for g, (dstA, dstB, gi, w1p) in enumerate(heads):
                nc.vector.tensor_sub(dstA[:, sl], t1s[g], t2s[g])
            t3s, t4s = [], []
            for g in range(3):
                t3 = tpool.tile([128, 256], F32, name=f"t3{g}", tag=f"hB{g}")
                nc.vector.tensor_mul(t3, qnB[g], csn[:, 1, :])
                t3s.append(t3)
                t4 = tpool.tile([128, 256], F32, name=f"t4{g}", tag=f"hC{g}")
                nc.gpsimd.tensor_mul(t4, qnA[g], csn[:, 3, :])
                t4s.append(t4)
            for g, (dstA, dstB, gi, w1p) in enumerate(heads):
                nc.vector.tensor_add(dstB[:, sl], t3s[g], t4s[g])

        def epilogue(ps, c, csn):
            epilogue_heads(c, csn, epilogue_stage(ps, c))

        def load_csn(c):
            # issued from the ACT queue: a pool-reuse wait here must not
            # stall the SP DMA stream feeding the matmuls
            sl = slice(c * 256, (c + 1) * 256)
            csn = cpool.tile([128, 4, 256], F32, name="csn", tag="csn")
            nc.scalar.dma_start(out=csn[:, 0:2, :], in_=cos_d[:, :, sl])
            nc.scalar.dma_start(out=csn[:, 2:4, :], in_=sin_d[:, :, sl])
            return csn

        # merged chunks 0+1 (paced by the startup DMA stream)
        csn0 = load_csn(0)
        csn1 = load_csn(1)
        ps0 = pps.tile([128, 8, 256], F32, name="ps0", tag="ps")
        ps1 = pps.tile([128, 8, 256], F32, name="ps1", tag="ps")
        # chunk 0/1 quarters all issued in the startup stream above; only
        # chunk 2's first quarters start here
        merged_loads = {15: [(2, 0)], 17: [(2, 1)]}
        for kt in range(KT):
            for (cc, qq) in merged_loads.get(kt, ()):
                load_hq(cc, qq)
            proj_matmuls(ps0, 0, kt)
            proj_matmuls(ps1, 1, kt)
            if kt % 5 == 4:
                free_hq(0, kt // 5)
                free_hq(1, kt // 5)
        psts0 = epilogue_stage(ps0, 0)
        epilogue_heads(0, csn0, psts0)
        psts1 = epilogue_stage(ps1, 1)
        heads_pending = (1, csn1, psts1)

        def chunk_loop(ps, c):
            for kt in range(KT):
                if kt == 0:
                    load_hq(c, 3)
                    if c == 2:
                        load_hq(c, 2)
                elif c + 1 < NC256:
                    if kt == 5:
                        load_hq(c + 1, 0)
                    elif kt == 10:
                        load_hq(c + 1, 1)
                    elif kt == 15:
                        load_hq(c + 1, 2)
                proj_matmuls(ps, c, kt)
                if kt % 5 == 4:
                    free_hq(c, kt // 5)

        for c in range(2, 8):
            csn = load_csn(c)
            ps = pps.tile([128, 8, 256], F32, name="ps", tag="ps")
            chunk_loop(ps, c)
            # previous chunk's norm/rope chain drains during this chunk's
            # matmuls; this chunk's PSUM staging lands right behind it
            epilogue_heads(*heads_pending)
            psts = epilogue_stage(ps, c)
            heads_pending = (c, csn, psts)
        pP.close()

        # phase-2 PSUM pools; the heads part of chunk 7's epilogue drains on
        # DVE/ACT/Pool while the first attention rounds start
        p2b = ExitStack()
        scps = p2b.enter_context(tc.tile_pool(name="scps", bufs=1, space="PSUM"))
        avps = p2b.enter_context(tc.tile_pool(name="avps", bufs=1, space="PSUM"))
        trps = p2b.enter_context(tc.tile_pool(name="trps", bufs=2, space="PSUM"))
        wops = p2b.enter_context(tc.tile_pool(name="wops", bufs=2, space="PSUM"))

        it_ctr = [0]

        def emit_scores_exp(t, h, early=False):
            """Scores + row-max + exp for (t, h); generator with stage marks.

            Early (single-chunk) rounds take their score bank from the
            still-idle wops pool so two rounds can pipeline before the main
            pools open."""
            r = it_ctr[0] % 2
            it_ctr[0] += 1
            w0, ch, pad = chunk_plan(t)
            assert not early or len(ch) == 1
            wt = min(t, 8) + 1
            if early:
                sc2 = wops.tile([128, 512], F32, name="scsE", tag="wop")
                chunk_ap = lambda ci: sc2
            else:
                scs = scps.tile([128, 3, 512], F32, name="scs", tag="scs")
                chunk_ap = lambda ci: scs[:, ci]
            nms = []
            off = 0
            for ci, nt in enumerate(ch):
                width = nt * 128
                ca = chunk_ap(ci)
                mlist = []
                if t >= 8 and ci == 0:
                    mlist.append((0, MK_BOUND))
                dlocal = (wt - 1) - off
                if 0 <= dlocal < nt:
                    mlist.append((dlocal, MK_DIAG))
                for p_ in range(nt):
                    if off + p_ >= wt:
                        mlist.append((p_, MK_FULL))
                first = True
                for (ti, kind) in mlist:
                    nc.tensor.matmul(
                        ca[:, ti * 128:(ti + 1) * 128],
                        identb, m3[:, kind, :], start=first, stop=False)
                    first = False
                kcol = (w0 + off) * 128
                for j in range(NDK):
                    nc.tensor.matmul(
                        ca[:, 0:width],
                        QT[2 * h + j][:, t * 128:(t + 1) * 128],
                        KTt[j][:, kcol:kcol + width],
                        start=(first and j == 0), stop=(j == NDK - 1))
                off += nt
                # row-max: batch the leading equal-width chunk pair
                if not early and ci == 1 and ch[0] == ch[1]:
                    nm = big.tile([128, 1], F32, name=f"nm0_{r}", tag=f"nm0_{r}")
                    nc.vector.tensor_reduce(out=nm, in_=scs[:, 0:2, 0:width],
                                            axis=AX.XY, op=AL.max, negate=True)
                    nms = [nm]
                elif not (not early and ci == 0 and len(ch) > 1 and ch[0] == ch[1]):
                    nm = big.tile([128, 1], F32, name=f"nm{ci}_{r}", tag=f"nm{ci}_{r}")
                    nc.vector.tensor_reduce(out=nm, in_=ca[:, 0:width],
                                            axis=AX.X, op=AL.max, negate=True)
                    nms.append(nm)
            yield "scores_done"
            negm = nms[0]
            if len(nms) > 1:
                negm = big.tile([128, 1], F32, name=f"negm_{r}", tag=f"negm_{r}")
                nc.vector.tensor_tensor(negm, nms[0], nms[1], op=AL.min)
                if len(nms) > 2:
                    nc.vector.tensor_tensor(negm, negm, nms[2], op=AL.min)
            yield "negm_done"
            if early:
                expf = big.tile([128, 384], F32R, name=f"expbE_{r}",
                                tag=f"expbE_{r}")
            else:
                expb = etpool.tile([128, 3, 384], F32R, name="expb", tag="expb")
                expf = expb.rearrange("p a b -> p (a b)")
            den = big.tile([128, 3], F32, name=f"den_{r}", tag=f"den_{r}")
            off = 0
            for ci, nt in enumerate(ch):
                width = nt * 128
                nc.scalar.activation(
                    out=expf[:, off * 128:off * 128 + width],
                    in_=chunk_ap(ci)[:, 0:width], func=AF.Exp, bias=negm,
                    accum_out=den[:, ci:ci + 1])
                off += nt
            if debug and t == 10 and h == 0:
                nc.sync.dma_start(out=dbg["dexpb"][:, 0:1152],
                                  in_=expf.bitcast(F32))
                nc.sync.dma_start(out=dbg["dnegm"][:, :], in_=negm)
                nc.sync.dma_start(out=dbg["dden"][:, :], in_=den)
            if len(ch) == 1:
                dsv = den[:, 0:1]
            else:
                dsum = big.tile([128, 1], F32, name=f"dsum_{r}", tag=f"dsum_{r}")
                nc.vector.tensor_reduce(out=dsum, in_=den[:, 0:len(ch)],
                                        axis=AX.X, op=AL.add)
                dsv = dsum
            rden = big.tile([128, 1], F32, name=f"rden_{r}", tag=f"rden_{r}")
            nc.vector.reciprocal(out=rden, in_=dsv)
            yield (expf, rden, ch, w0)

        def run_gen(g, until):
            while True:
                v = next(g)
                if v == until or not isinstance(v, str):
                    return v

        cp_rr = [0]

        def psum_copy(dst, src):
            e = (0, 1)[cp_rr[0] % 2]      # 1:1 DVE:ACT
            cp_rr[0] += 1
            if e == 0:
                nc.vector.tensor_copy(out=dst, in_=src)
            else:
                nc.scalar.copy(out=dst, in_=src)

        def emit_attn_core(state, trpool, avpool, expT, ao_dst):
            """Transposes, AV matmul, and attn-out scale into ao_dst."""
            expf, rden, ch, w0 = state
            T = sum(ch)
            expTf = expT.rearrange("p a b -> p (a b)")
            off = 0
            for ci, nt in enumerate(ch):
                width = nt * 128
                trp = trpool.tile([128, 512], F32R, name="trp", tag="trp")
                for q_ in range(nt):
                    nc.tensor.transpose(
                        trp[:, q_ * 128:(q_ + 1) * 128],
                        expf[:, (off + q_) * 128:(off + q_ + 1) * 128], ident)
                psum_copy(expTf[:, off * 128:off * 128 + width], trp[:, 0:width])
                off += nt
            avp = avpool.tile([128, D], F32, name="avp", tag="avp")
            for a in range(T):
                nc.tensor.matmul(avp, expT[:, a, :], V[:, w0 + a, :],
                                 start=(a == 0), stop=(a == T - 1))
            nc.scalar.activation(out=ao_dst, in_=avp, func=AF.Copy, scale=rden)

        # boundary rounds t=0..NEARLY-1: scores/exp/AV run while the chunk-7
        # heads chain drains; attn-out goes to big-pool slots, aoT deferred
        first = emit_scores_exp(0, 0, early=True)
        run_gen(first, "scores_done")
        st00 = run_gen(first, None)
        emit_attn_core(st00, trps, avps, expTE, aoE[0])
        epilogue_heads(*heads_pending)
        for t in range(NEARLY):
            for h in range(2):
                if t == 0 and h == 0:
                    continue
                g = emit_scores_exp(t, h, early=True)
                run_gen(g, "scores_done")
                st = run_gen(g, None)
                emit_attn_core(st, trps, avps, expTE, aoE[2 * t + h])
        pA.close()

        # ---------------- Phase 2 main + output projection ----------------
        etpool = p2b.enter_context(tc.tile_pool(name="etpool", bufs=2))
        wopool = p2b.enter_context(tc.tile_pool(name="wopool", bufs=1))
        aopool = p2b.enter_context(tc.tile_pool(name="aopool", bufs=1))
        opool = p2b.enter_context(tc.tile_pool(name="opool", bufs=2))

        woT = wopool.tile([128, NDQ, H], F32R, tag="woT")
        for dj in range(NDQ):
            nc.sync.dma_start(out=woT[:, dj:dj + 1, :], in_=wo_d[:, dj:dj + 1, :])
        aoT = [aopool.tile([128, S], F32R, name=f"aoT{j}", tag=f"aoT{j}")
               for j in range(NDQ)]

        wo_queue = []
        ost_tiles = {}

        def emit_wo_part(min_backlog=0):
            """Emit one Wo output column chunk (4 matmuls + copy) from the
            backlog; fine granules let PE fill sub-us dependency holes."""
            if len(wo_queue) <= min_backlog:
                return
            t, hc = wo_queue.pop(0)
            if hc == 0:
                ost_tiles[t] = opool.tile([128, H], F32, name="ost", tag="ost")
            ost = ost_tiles[t]
            wop = wops.tile([128, 512], F32, name="wop", tag="wop")
            for dj in range(NDQ):
                nc.tensor.matmul(
                    wop, aoT[dj][:, t * 128:(t + 1) * 128],
                    woT[:, dj, hc * 512:(hc + 1) * 512],
                    start=(dj == 0), stop=(dj == NDQ - 1))
            psum_copy(ost[:, hc * 512:(hc + 1) * 512], wop)
            if hc == 2:
                nc.sync.dma_start(
                    out=out_d[t * 128:(t + 1) * 128, 0:1536],
                    in_=ost[:, 0:1536])
            elif hc == 4:
                nc.sync.dma_start(
                    out=out_d[t * 128:(t + 1) * 128, 1536:H],
                    in_=ost_tiles.pop(t)[:, 1536:H])

        def emit_aoT(t, h, ao_src):
            trp = trps.tile([128, 512], F32R, name="trpa", tag="trp")
            for j in range(2):
                nc.tensor.transpose(trp[:, j * 128:(j + 1) * 128],
                                    ao_src[:, j * 128:(j + 1) * 128], ident)
            for j in range(2):
                psum_copy(aoT[2 * h + j][:, t * 128:(t + 1) * 128],
                          trp[:, j * 128:(j + 1) * 128])

        # catch up: aoT for the early rounds, then queue their Wo parts
        for t in range(NEARLY):
            for h in range(2):
                emit_aoT(t, h, aoE[2 * t + h])
            wo_queue += [(t, hc) for hc in range(5)]
            if t >= 2:
                emit_wo_part()
                emit_wo_part()

        for t in range(NEARLY, ST):
            for h in range(2):
                g = emit_scores_exp(t, h)
                run_gen(g, "scores_done")
                emit_wo_part()
                emit_wo_part(min_backlog=4)
                run_gen(g, "negm_done")
                emit_wo_part(min_backlog=2)
                state = run_gen(g, None)
                ao = etpool.tile([128, D], F32R, name="ao", tag="ao")
                expT = etpool.tile([128, 9, 128], F32R, name="expT", tag="expT")
                emit_attn_core(state, trps, avps, expT, ao)
                emit_wo_part(min_backlog=6)
                emit_aoT(t, h, ao)
            wo_queue += [(t, hc) for hc in range(5)]
            emit_wo_part(min_backlog=8)

        while wo_queue:
            emit_wo_part()

        p2b.close()

    nc.compile()
    return nc


_nc_cache = None


def _prep_inputs(hidden_states, attention_mask, cos, sin, Wq, Wk, Wv, Wo,
                 q_norm_w, k_norm_w):
    import ml_dtypes
    bf16 = ml_dtypes.bfloat16

    hidden_states = np.asarray(hidden_states, dtype=np.float32)
    mask = np.asarray(attention_mask, dtype=np.float32)[0, 0]      # [S, S]
    cos2 = np.asarray(cos, dtype=np.float32)[0, 0]                 # [S, D]
    sin2 = np.asarray(sin, dtype=np.float32)[0, 0]
    Wq = np.asarray(Wq, dtype=np.float32)
    Wk = np.asarray(Wk, dtype=np.float32)
    Wv = np.asarray(Wv, dtype=np.float32)
    Wo = np.asarray(Wo, dtype=np.float32)

    cosh = np.ascontiguousarray(
        cos2.T.reshape(NDK, 128, S).transpose(1, 0, 2))            # [128,2,S]
    sinh = np.ascontiguousarray(sin2.T.reshape(NDK, 128, S).transpose(1, 0, 2))

    diag = mask[0:128, 0:128]
    bound = mask[1024:1152, 0:128]
    full = mask[0:128, 1024:1152]
    m3h = np.ascontiguousarray(
        np.stack([diag, bound, full], axis=1)).astype(bf16)        # [128,3,128]

    qw1p = np.ascontiguousarray(
        (1.0 + np.asarray(q_norm_w, dtype=np.float32)).reshape(NDK, 128).T)
    kw1p = np.ascontiguousarray(
        (1.0 + np.asarray(k_norm_w, dtype=np.float32)).reshape(NDK, 128).T)
    ident = round_f32r(np.eye(128, dtype=np.float32))
    identb = np.eye(128, dtype=np.float32).astype(bf16)

    in_maps = []
    for core in range(8):
        b, g = core // 4, core % 4
        hsT = round_f32r(np.ascontiguousarray(hidden_states[b].T))  # [H, S]
        hsC = np.ascontiguousarray(
            hsT.reshape(KT, 128, NC256, 256).transpose(2, 1, 0, 3))
        wqh = round_f32r(np.ascontiguousarray(
            Wq[2 * g * D:(2 * g + 2) * D].T.reshape(KT, 128, DQ).transpose(1, 0, 2)))
        wkh = round_f32r(np.ascontiguousarray(
            Wk[g * D:(g + 1) * D].T.reshape(KT, 128, D).transpose(1, 0, 2)))
        wvh = round_f32r(np.ascontiguousarray(
            Wv[g * D:(g + 1) * D].T.reshape(KT, 128, D).transpose(1, 0, 2)))
        woh = round_f32r(np.ascontiguousarray(
            Wo[:, 2 * g * D:(2 * g + 2) * D].T.reshape(NDQ, 128, H).transpose(1, 0, 2)))
        w0pack = np.ascontiguousarray(
            np.concatenate([wqh[:, 0, :], wkh[:, 0, :], wvh[:, 0, :]],
                           axis=1).reshape(128, 4, 256))
        in_maps.append({
            "hsC": hsC, "wqh": wqh, "wkh": wkh, "wvh": wvh, "woh": woh,
            "cosh": cosh, "sinh": sinh, "m3h": m3h, "ident": ident,
            "identb": identb, "qw1p": qw1p, "kw1p": kw1p, "w0pack": w0pack,
        })
    return in_maps


def kernel(hidden_states, attention_mask, cos, sin, Wq, Wk, Wv, Wo,
           q_norm_w, k_norm_w):
    global _nc_cache
    if _nc_cache is None:
        _nc_cache = build_nc()
    nc = _nc_cache

    in_maps = _prep_inputs(hidden_states, attention_mask, cos, sin,
                           Wq, Wk, Wv, Wo, q_norm_w, k_norm_w)
    res = run_bass_kernel_spmd(nc, in_maps, core_ids=list(range(8)))
    outs = [r["out"] for r in res.results]
    final = np.zeros((B, S, H), dtype=np.float32)
    for core in range(8):
        final[core // 4] += outs[core]
    return final
